# revision 4
# baseline (speedup 1.0000x reference)
"""Causal single-head attention layer on 8 TRN2 NeuronCores.

Problem: X[4,2048,1024]; Q/K/V = X@W+b; scores = Q@K^T (no 1/sqrt(d));
causal mask; softmax; out = P@V.

Sharding: 2 cores per batch. Each core owns 8 query tiles (128 rows) of
its batch, folded for causal load balance:
  core h=0 -> global q-tiles (0,3,4,7,8,11,12,15)
  core h=1 -> global q-tiles (1,2,5,6,9,10,13,14)
Slot s on either core has causal extent <= 2s+2 k-tiles, so ONE uniform
program runs on all 8 cores; the exact causal boundary is a host-supplied
0/1 mask over the last two k-tiles of each slot.

Math restructuring:
  scores = (XqWq+bq)(XkWk+bk)^T
         = Xq G Xk^T + [q-only term] + w[k] + [const],  G = Wq Wk^T (host)
  q-only and const terms cancel in softmax; w[k] = Xk @ (Wk bq) (host)
  rides the per-partition bias slot of the Exp activation.
  The V projection is folded PAST the attention sum (associativity):
    out = (E @ Xk) @ Wv / rowsum + bv = U @ Wv / rowsum + bv
  so the [S,D]x[D,D] V projection (duplicated on both cores of a batch)
  is replaced by a per-core [1024,D]x[D,D] output projection.

On-device layout (contraction always on partitions):
  host passes X^T (xkt, f32r) for scores, X (xkd, bf16) for the U
  accumulation; scores^T[k,q] accumulate fp32 in PSUM; E = exp(scores^T
  + w[k]) in bf16 (w rides the Exp bias slot); U^T[d,q] = sum_k Xk^T E
  accumulates per d-tile in PSUM with causally narrowed moving dims
  (bf16 has no N>=256 restriction; fp32r scores clamp at N=256);
  row sums via matmul with ones; out[q,d] = (U@Wv)/sums + bv, Wv bf16.
  No max-subtraction needed: |scores| <= ~60 so exp stays in range.

Scheduling notes (PE pstate: any PE idle gap drops the clock for ~3us,
so the program is ordered to keep PE streaks long):
  - One flat PSUM pool set spans Qg + attention (Qg chains ride the U
    pool's ring) so there is no PSUM pool-transition barrier between
    the Qg copies and the first score matmul.
  - Qg runs as per-(do,c) chains, all c=0 chains first: their copies
    trail one chain behind, and scores of block 0 (which read only the
    c=0 half of Qg) start right after the c=1 chains with no wait.
  - DMA order = consumption order: (g|xq interleaved), xkt k-blocks
    0-1, wb+mask, xkd tiles 0-7, xkt k-blocks 2-3, xkd 8-15, wv, bvp.
  - PE order: Qg-c0, Qg-c1, [b0 scores, U, rowsums], [b1 scores],
    [b0 out], [b1 U, rowsums, out].
"""

import numpy as np
import ml_dtypes

import concourse.bass as bass  # noqa: F401
import concourse.mybir as mybir
from concourse import bacc
from concourse.bass_utils import run_bass_kernel_spmd
from concourse.tile import TileContext

F32 = mybir.dt.float32
F32R = mybir.dt.float32r
BF16 = mybir.dt.bfloat16
EXP = mybir.ActivationFunctionType.Exp

B, S, D = 4, 2048, 1024
P = 128
DT = D // P          # 8 d-tiles
QT = 8               # q-tile slots per core
KT = S // P          # 16 k-tiles
EXT = [2 * s + 2 for s in range(QT)]   # uniform per-slot k-extent
BLK = [(0, 4, 8), (4, 8, 16)]          # (slot_lo, slot_hi, block k-extent)

QTS = {0: [0, 3, 4, 7, 8, 11, 12, 15], 1: [1, 2, 5, 6, 9, 10, 13, 14]}

_CACHE = {}


def _build(reps=1):
    nc = bacc.Bacc("TRN2", target_bir_lowering=False, debug=False, num_devices=8)
    xqt = nc.declare_dram_parameter("xqt", [D, QT * P], F32R, isOutput=False)
    xkt = nc.declare_dram_parameter("xkt", [D, S], F32R, isOutput=False)
    g = nc.declare_dram_parameter("g", [D, D], F32R, isOutput=False)
    xkd = nc.declare_dram_parameter("xkd", [S, D], BF16, isOutput=False)
    wv = nc.declare_dram_parameter("wv", [D, D], BF16, isOutput=False)
    wb = nc.declare_dram_parameter("wb", [P, KT], F32, isOutput=False)
    bvp = nc.declare_dram_parameter("bvp", [P, D], F32, isOutput=False)
    msk = nc.declare_dram_parameter("msk", [P, QT * 2 * P], BF16, isOutput=False)
    y = nc.declare_dram_parameter("y", [QT * P, D], F32, isOutput=True)

    with TileContext(nc) as tc:
      for _rep in range(reps):
        with tc.tile_pool(name="persist", bufs=1) as pp:
            # ---- persistent tiles ----
            xk_sb = [pp.tile([P, S], F32R, tag=f"xk{i}", name=f"xk{i}")
                     for i in range(DT)]
            qg_sb = [pp.tile([P, QT * P], F32R, tag=f"qg{i}", name=f"qg{i}")
                     for i in range(DT)]
            xkd_sb = [pp.tile([P, D], BF16, tag=f"xkd{i}", name=f"xkd{i}")
                      for i in range(KT)]
            wb_sb = pp.tile([P, KT], F32, tag="wb", name="wb_sb")
            bv_sb = pp.tile([P, D], F32, tag="bv", name="bv_sb")
            mask_sb = pp.tile([P, QT * 2 * P], BF16, tag="mask", name="mask_sb")
            ones_sb = pp.tile([P, 1], BF16, tag="ones", name="ones_sb")

            with (
                tc.tile_pool(name="pssc", bufs=2, space="PSUM") as ps_s,
                tc.tile_pool(name="psu", bufs=4, space="PSUM") as ps_u,
                tc.tile_pool(name="pso", bufs=2, space="PSUM") as ps_o,
            ):
                # ---- Qg phase: Qg^T[d2,q] = sum_d1 G[d1,d2] Xq^T[d1,q] ----
                with tc.tile_pool(name="qgpool", bufs=1) as qp:
                    xq_sb = [qp.tile([P, QT * P], F32R, tag=f"xq{i}",
                                     name=f"xq{i}") for i in range(DT)]
                    g_sb = {}
                    # chain (do,c) consumes xq[dd] in dd order, so issue
                    # g-block[do=dd] right after xq[dd].
                    for dd in range(DT):
                        nc.sync.dma_start(out=xq_sb[dd][:],
                                          in_=xqt[dd * P:(dd + 1) * P, :])
                        g_sb[dd] = [qp.tile([P, P], F32R, tag=f"g{dd}_{r}",
                                            name=f"g{dd}_{r}")
                                    for r in range(DT)]
                        for r in range(DT):
                            nc.sync.dma_start(
                                out=g_sb[dd][r][:],
                                in_=g[r * P:(r + 1) * P, dd * P:(dd + 1) * P])
                    # attention inputs stream in behind the Qg inputs, in
                    # consumption order (k-block-major for the scores).
                    for kb in range(2):
                        for dd in range(DT):
                            nc.sync.dma_start(
                                out=xk_sb[dd][:, kb * 512:(kb + 1) * 512],
                                in_=xkt[dd * P:(dd + 1) * P,
                                        kb * 512:(kb + 1) * 512])
                    nc.sync.dma_start(out=wb_sb[:], in_=wb[:])
                    nc.sync.dma_start(out=mask_sb[:], in_=msk[:])
                    for kt in range(8):
                        nc.sync.dma_start(out=xkd_sb[kt][:],
                                          in_=xkd[kt * P:(kt + 1) * P, :])
                    for kb in range(2, 4):
                        for dd in range(DT):
                            nc.sync.dma_start(
                                out=xk_sb[dd][:, kb * 512:(kb + 1) * 512],
                                in_=xkt[dd * P:(dd + 1) * P,
                                        kb * 512:(kb + 1) * 512])
                    for kt in range(8, KT):
                        nc.sync.dma_start(out=xkd_sb[kt][:],
                                          in_=xkd[kt * P:(kt + 1) * P, :])
                    nc.sync.dma_start(out=bv_sb[:], in_=bvp[:])
                    nc.gpsimd.memset(ones_sb[:], 1.0)

                    # Qg chains ride the psu ring: no fresh PSUM pool, so
                    # the first score matmul has no pool-transition wait.
                    for c in range(2):
                        for do in range(DT):
                            pq = ps_u.tile([P, 512], F32, tag="pu",
                                           name=f"pq{c}_{do}")
                            for dd in range(DT):
                                nc.tensor.matmul(
                                    pq[:],
                                    g_sb[do][dd][:],
                                    xq_sb[dd][:, c * 512:(c + 1) * 512],
                                    start=(dd == 0), stop=(dd == DT - 1),
                                )
                            nc.vector.tensor_copy(
                                qg_sb[do][:, c * 512:(c + 1) * 512], pq[:])

                # ---- Attention ----
                with (
                    tc.tile_pool(name="attn", bufs=1) as ap,
                    tc.tile_pool(name="estage", bufs=24) as ep,
                    tc.tile_pool(name="ostage", bufs=2) as op,
                    tc.tile_pool(name="small", bufs=4) as sp,
                ):
                    wv_sb = [ap.tile([P, D], BF16, tag=f"wvx{i}",
                                     name=f"wvx{i}") for i in range(DT)]
                    ut_sb = [ap.tile([P, QT * P], BF16, tag=f"ut{i}",
                                     name=f"ut{i}") for i in range(DT)]
                    for dd in range(DT):
                        nc.sync.dma_start(out=wv_sb[dd][:],
                                          in_=wv[dd * P:(dd + 1) * P, :])

                    def scores_block(s0, s1, bext, q0):
                        e_tiles, e_offs, u_offs = [], [], []
                        for kt in range(bext):
                            # slots below ls_min never read k-tile kt
                            # (causal): narrow the moving dim, fp32r keeps
                            # N >= 256 (below which it slows 4x).
                            ls_min = kt // 2
                            offu = max(0, ls_min - s0) * P
                            offs = min(offu, 256)
                            n = 512 - offs
                            pscore = ps_s.tile([P, 512], F32, tag="sc")
                            for dd in range(DT):
                                nc.tensor.matmul(
                                    pscore[:, 0:n],
                                    xk_sb[dd][:, kt * P:(kt + 1) * P],
                                    qg_sb[dd][:, q0 + offs:q0 + 512],
                                    start=(dd == 0), stop=(dd == DT - 1),
                                )
                            et = ep.tile([P, 512], BF16, tag="E")
                            # E = exp(scores^T + w[k]) (w in the bias slot)
                            nc.scalar.activation(et[:, 0:n], pscore[:, 0:n],
                                                 EXP, bias=wb_sb[:, kt:kt + 1])
                            e_tiles.append(et)
                            e_offs.append(offs)
                            u_offs.append(offu)
                            # causal boundary mask on each slot's two
                            # diagonal k-tiles, applied eagerly so the U
                            # accumulation can batch whole q-blocks.
                            for ls in range(s0, s1):
                                if kt == EXT[ls] - 2 or kt == EXT[ls] - 1:
                                    j = kt - (EXT[ls] - 2)
                                    lo = (ls - s0) * P - offs
                                    nc.vector.tensor_mul(
                                        et[:, lo:lo + P],
                                        et[:, lo:lo + P],
                                        mask_sb[:, (2 * ls + j) * P:
                                                (2 * ls + j + 1) * P],
                                    )
                        return e_tiles, e_offs, u_offs

                    def u_block(bext, q0, e_tiles, e_offs, u_offs):
                        # U^T[d,q] = sum_k Xk[k,d]^T E[k,q].  Column ranges
                        # narrow monotonically with kt and nest inside the
                        # kt=0 full-width start=True write, so each column
                        # accumulates exactly its causal k-extent.
                        for dt in range(DT):
                            pu = ps_u.tile([P, 512], F32, tag="pu",
                                           name=f"pu{dt}")
                            for kt in range(bext):
                                ou, os_ = u_offs[kt], e_offs[kt]
                                nc.tensor.matmul(
                                    pu[:, ou:512],
                                    xkd_sb[kt][:, dt * P:(dt + 1) * P],
                                    e_tiles[kt][:, ou - os_:512 - os_],
                                    start=(kt == 0), stop=(kt == bext - 1),
                                )
                            nc.vector.tensor_copy(
                                ut_sb[dt][:, q0:q0 + 512], pu[:])

                    def rowsums_block(s0, s1, e_tiles, e_offs):
                        rcs = {}
                        for ls in range(s0, s1):
                            lq = (ls - s0) * P
                            pm = ps_o.tile([P, 1], F32, tag="po",
                                           name=f"pm{ls}")
                            for kt in range(EXT[ls]):
                                el = e_tiles[kt][:, lq - e_offs[kt]:
                                                 lq - e_offs[kt] + P]
                                nc.tensor.matmul(pm[:], el, ones_sb[:],
                                                 start=(kt == 0),
                                                 stop=(kt == EXT[ls] - 1))
                            rc = sp.tile([P, 1], F32, tag="rc",
                                         name=f"rc{ls}")
                            nc.vector.reciprocal(rc[:], pm[:])
                            rcs[ls] = rc
                        return rcs

                    def out_block(s0, s1, rcs):
                        # out[q,d] = (U @ Wv) / rowsum + bv
                        for ls in range(s0, s1):
                            ot = op.tile([P, D], F32, tag="ot")
                            for j in range(2):
                                po = ps_o.tile([P, 512], F32, tag="po",
                                               name=f"po{ls}_{j}")
                                for dt in range(DT):
                                    nc.tensor.matmul(
                                        po[:],
                                        ut_sb[dt][:, ls * P:(ls + 1) * P],
                                        wv_sb[dt][:, j * 512:(j + 1) * 512],
                                        start=(dt == 0), stop=(dt == DT - 1),
                                    )
                                nc.vector.tensor_scalar_mul(
                                    ot[:, j * 512:(j + 1) * 512], po[:],
                                    rcs[ls][:])
                                nc.vector.tensor_add(
                                    ot[:, j * 512:(j + 1) * 512],
                                    ot[:, j * 512:(j + 1) * 512],
                                    bv_sb[:, j * 512:(j + 1) * 512])
                                nc.sync.dma_start(
                                    out=y[ls * P:(ls + 1) * P,
                                          j * 512:(j + 1) * 512],
                                    in_=ot[:, j * 512:(j + 1) * 512])

                    s0, s1, bext = BLK[0]
                    e0, eo0, uo0 = scores_block(s0, s1, bext, s0 * P)
                    u_block(bext, s0 * P, e0, eo0, uo0)
                    rcs0 = rowsums_block(s0, s1, e0, eo0)

                    t0, t1, bext1 = BLK[1]
                    e1, eo1, uo1 = scores_block(t0, t1, bext1, t0 * P)
                    out_block(s0, s1, rcs0)
                    u_block(bext1, t0 * P, e1, eo1, uo1)
                    rcs1 = rowsums_block(t0, t1, e1, eo1)
                    out_block(t0, t1, rcs1)

    nc.compile()
    return nc


def _get_nc():
    if "nc" not in _CACHE:
        _CACHE["nc"] = _build()
    return _CACHE["nc"]


def make_in_maps(X, Wq, bq, Wk, bk, Wv, bv):
    X = np.asarray(X, np.float32)
    Wq = np.asarray(Wq, np.float32)
    Wk = np.asarray(Wk, np.float32)
    Wv = np.ascontiguousarray(np.asarray(Wv, np.float32))
    bq = np.asarray(bq, np.float32)
    bv = np.asarray(bv, np.float32)

    G = np.ascontiguousarray(Wq @ Wk.T)          # [D, D]
    wkbq = Wk @ bq                               # [D]
    bvp = np.ascontiguousarray(np.broadcast_to(bv[None, :], (P, D)))
    wv16 = Wv.astype(ml_dtypes.bfloat16)

    masks = {}
    for h in (0, 1):
        m = np.zeros((QT, 2 * P, P), np.float32)
        for s in range(QT):
            qt = QTS[h][s]
            kk = (2 * s) * P + np.arange(2 * P)[:, None]
            qq = qt * P + np.arange(P)[None, :]
            m[s] = (kk <= qq)
        # [s, kk, q] -> [kk%128, s*256 + (kk//128)*128 + q]
        m2 = m.reshape(QT, 2, P, P).transpose(2, 0, 1, 3).reshape(P, QT * 2 * P)
        masks[h] = np.ascontiguousarray(m2.astype(ml_dtypes.bfloat16))

    in_maps = []
    for c in range(8):
        b, h = divmod(c, 2)
        Xb = X[b]
        xkt = np.ascontiguousarray(Xb.T)
        xkd = np.ascontiguousarray(Xb.astype(ml_dtypes.bfloat16))
        xq_rows = np.concatenate(
            [Xb[qt * P:(qt + 1) * P] for qt in QTS[h]], axis=0)
        xqt = np.ascontiguousarray(xq_rows.T)
        w = Xb @ wkbq                             # [S] additive k-bias
        wbp = np.ascontiguousarray(w.reshape(KT, P).T)   # [P, KT]
        in_maps.append({
            "xqt": xqt, "xkt": xkt, "g": G, "xkd": xkd, "wv": wv16,
            "wb": wbp, "bvp": bvp, "msk": masks[h],
        })
    return in_maps


def assemble(results):
    Y = np.empty((B, S, D), np.float32)
    for c in range(8):
        b, h = divmod(c, 2)
        yc = results[c]["y"]
        for s in range(QT):
            qt = QTS[h][s]
            Y[b, qt * P:(qt + 1) * P, :] = yc[s * P:(s + 1) * P, :]
    return Y


def kernel(X, Wq, bq, Wk, bk, Wv, bv):
    nc = _get_nc()
    in_maps = make_in_maps(X, Wq, bq, Wk, bk, Wv, bv)
    res = run_bass_kernel_spmd(nc, in_maps, core_ids=list(range(8)))
    return assemble(res.results)


# revision 9
# speedup vs baseline: 1.1601x; 1.1601x over previous
"""Causal single-head attention layer on 8 TRN2 NeuronCores.

Problem: X[4,2048,1024]; Q/K/V = X@W+b; scores = Q@K^T (no 1/sqrt(d));
causal mask; softmax; out = P@V.

Sharding: 2 cores per batch. Each core owns 8 query tiles (128 rows) of
its batch, folded for causal load balance:
  core h=0 -> global q-tiles (0,3,4,7,8,11,12,15)
  core h=1 -> global q-tiles (1,2,5,6,9,10,13,14)
Slot s on either core has causal extent <= 2s+2 k-tiles, so ONE uniform
program runs on all 8 cores; the exact causal boundary is a host-supplied
0/1 mask over the last two k-tiles of each slot.

Math restructuring:
  scores = (XqWq+bq)(XkWk+bk)^T
         = Xq G Xk^T + [q-only term] + w[k] + [const],  G = Wq Wk^T (host)
  q-only and const terms cancel in softmax; w[k] = Xk @ (Wk bq) (host)
  rides the per-partition bias slot of the Exp activation.
  The V projection is folded PAST the attention sum (associativity):
    out = (E @ Xk) @ Wv / rowsum + bv = U @ Wv / rowsum + bv
  so the [S,D]x[D,D] V projection (duplicated on both cores of a batch)
  is replaced by a per-core [1024,D]x[D,D] output projection.

On-device layout (contraction always on partitions):
  host passes X^T (xkt, f32r) for scores, X (xkd, bf16) for the U
  accumulation; scores^T[k,q] accumulate fp32 in PSUM; E = exp(scores^T
  + w[k]) in bf16 (w rides the Exp bias slot); U^T[d,q] = sum_k Xk^T E
  accumulates per d-tile in PSUM with causally narrowed moving dims
  (bf16 has no N>=256 restriction; fp32r scores clamp at N=256);
  row sums via matmul with ones; out[q,d] = (U@Wv)/sums + bv, Wv bf16.
  No max-subtraction needed: |scores| <= ~60 so exp stays in range.

Scheduling notes (PE pstate: any PE idle gap drops the clock for ~3us,
so the program is ordered to keep PE streaks long):
  - One flat PSUM pool set spans Qg + attention (Qg chains ride the U
    pool's ring) so there is no PSUM pool-transition barrier between
    the Qg copies and the first score matmul.
  - Qg runs as per-(do,c) chains, all c=0 chains first: their copies
    trail one chain behind, and scores of block 0 (which read only the
    c=0 half of Qg) start right after the c=1 chains with no wait.
  - DMA order = consumption order: (g|xq interleaved), xkt k-blocks
    0-1, wb+mask, xkd tiles 0-7, xkt k-blocks 2-3, xkd 8-15, wv, bvp.
  - PE order: Qg-c0, Qg-c1, [b0 scores, U, rowsums], [b1 scores],
    [b0 out], [b1 U, rowsums, out].
"""

import numpy as np
import ml_dtypes

import concourse.bass as bass  # noqa: F401
import concourse.mybir as mybir
from concourse import bacc
from concourse.bass_utils import run_bass_kernel_spmd
from concourse.tile import TileContext

F32 = mybir.dt.float32
F32R = mybir.dt.float32r
BF16 = mybir.dt.bfloat16
EXP = mybir.ActivationFunctionType.Exp

B, S, D = 4, 2048, 1024
P = 128
DT = D // P          # 8 d-tiles
QT = 8               # q-tile slots per core
KT = S // P          # 16 k-tiles
EXT = [2 * s + 2 for s in range(QT)]   # uniform per-slot k-extent
BLK = [(0, 4, 8), (4, 8, 16)]          # (slot_lo, slot_hi, block k-extent)

QTS = {0: [0, 3, 4, 7, 8, 11, 12, 15], 1: [1, 2, 5, 6, 9, 10, 13, 14]}

_CACHE = {}


def _build(reps=1):
    nc = bacc.Bacc("TRN2", target_bir_lowering=False, debug=False, num_devices=8)
    xqt = nc.declare_dram_parameter("xqt", [D, QT * P], F32R, isOutput=False)
    xkt = nc.declare_dram_parameter("xkt", [D, S], F32R, isOutput=False)
    g = nc.declare_dram_parameter("g", [P, DT * D], F32R, isOutput=False)
    xkd = nc.declare_dram_parameter("xkd", [S, D], BF16, isOutput=False)
    wv = nc.declare_dram_parameter("wv", [D, D], BF16, isOutput=False)
    wb = nc.declare_dram_parameter("wb", [P, KT], F32, isOutput=False)
    bvp = nc.declare_dram_parameter("bvp", [P, D], F32, isOutput=False)
    msk = nc.declare_dram_parameter("msk", [P, QT * 2 * P], BF16, isOutput=False)
    y = nc.declare_dram_parameter("y", [QT * P, D], F32, isOutput=True)

    with TileContext(nc) as tc:
      for _rep in range(reps):
        with tc.tile_pool(name="persist", bufs=1) as pp:
            # ---- persistent tiles ----
            xk_sb = [pp.tile([P, S], F32R, tag=f"xk{i}", name=f"xk{i}")
                     for i in range(DT)]
            qg_sb = [pp.tile([P, QT * P], F32R, tag=f"qg{i}", name=f"qg{i}")
                     for i in range(DT)]
            xkd_sb = [pp.tile([P, D], BF16, tag=f"xkd{i}", name=f"xkd{i}")
                      for i in range(KT)]
            wb_sb = pp.tile([P, KT], F32, tag="wb", name="wb_sb")
            bv_sb = pp.tile([P, D], F32, tag="bv", name="bv_sb")
            mask_sb = pp.tile([P, QT * 2 * P], BF16, tag="mask", name="mask_sb")
            ones_sb = pp.tile([P, 1], BF16, tag="ones", name="ones_sb")

            with (
                tc.tile_pool(name="pssc", bufs=2, space="PSUM") as ps_s,
                tc.tile_pool(name="psu", bufs=4, space="PSUM") as ps_u,
                tc.tile_pool(name="pso", bufs=2, space="PSUM") as ps_o,
            ):
                # ---- Qg phase: Qg^T[d2,q] = sum_d1 G[d1,d2] Xq^T[d1,q] ----
                with tc.tile_pool(name="qgpool", bufs=1) as qp:
                    xq_sb = [qp.tile([P, QT * P], F32R, tag=f"xq{i}",
                                     name=f"xq{i}") for i in range(DT)]
                    # g_sb[do] cols dd*128+c hold G[dd*128+p, do*128+c]
                    # (host-retiled so each do-block is one large DMA).
                    g_sb = [qp.tile([P, D], F32R, tag=f"g{i}", name=f"g{i}")
                            for i in range(DT)]
                    # c=0 chains need only the first halves of xq, so land
                    # those, then the g blocks, then the second halves.
                    for dd in range(DT):
                        nc.sync.dma_start(out=xq_sb[dd][:, 0:512],
                                          in_=xqt[dd * P:(dd + 1) * P, 0:512])
                    for do in range(DT):
                        nc.sync.dma_start(out=g_sb[do][:],
                                          in_=g[:, do * D:(do + 1) * D])
                    for dd in range(DT):
                        nc.sync.dma_start(out=xq_sb[dd][:, 512:1024],
                                          in_=xqt[dd * P:(dd + 1) * P,
                                                  512:1024])
                    # attention inputs stream in behind the Qg inputs, in
                    # consumption order (k-block-major for the scores).
                    for kb in range(2):
                        for dd in range(DT):
                            nc.sync.dma_start(
                                out=xk_sb[dd][:, kb * 512:(kb + 1) * 512],
                                in_=xkt[dd * P:(dd + 1) * P,
                                        kb * 512:(kb + 1) * 512])
                    nc.sync.dma_start(out=wb_sb[:], in_=wb[:])
                    nc.sync.dma_start(out=mask_sb[:], in_=msk[:])
                    for kt in range(8):
                        nc.sync.dma_start(out=xkd_sb[kt][:],
                                          in_=xkd[kt * P:(kt + 1) * P, :])
                    for kb in range(2, 4):
                        for dd in range(DT):
                            nc.sync.dma_start(
                                out=xk_sb[dd][:, kb * 512:(kb + 1) * 512],
                                in_=xkt[dd * P:(dd + 1) * P,
                                        kb * 512:(kb + 1) * 512])
                    for kt in range(8, KT):
                        nc.sync.dma_start(out=xkd_sb[kt][:],
                                          in_=xkd[kt * P:(kt + 1) * P, :])
                    nc.sync.dma_start(out=bv_sb[:], in_=bvp[:])
                    nc.gpsimd.memset(ones_sb[:], 1.0)

                    # Qg chains ride the psu ring: no fresh PSUM pool, so
                    # the first score matmul has no pool-transition wait.
                    for c in range(2):
                        for do in range(DT):
                            pq = ps_u.tile([P, 512], F32, tag="pu",
                                           name=f"pq{c}_{do}")
                            for dd in range(DT):
                                nc.tensor.matmul(
                                    pq[:],
                                    g_sb[do][:, dd * P:(dd + 1) * P],
                                    xq_sb[dd][:, c * 512:(c + 1) * 512],
                                    start=(dd == 0), stop=(dd == DT - 1),
                                )
                            nc.vector.tensor_copy(
                                qg_sb[do][:, c * 512:(c + 1) * 512], pq[:])

                # ---- Attention ----
                with (
                    tc.tile_pool(name="attn", bufs=1) as ap,
                    tc.tile_pool(name="estage", bufs=24) as ep,
                    tc.tile_pool(name="ostage", bufs=2) as op,
                    tc.tile_pool(name="small", bufs=4) as sp,
                ):
                    wv_sb = [ap.tile([P, D], BF16, tag=f"wvx{i}",
                                     name=f"wvx{i}") for i in range(DT)]
                    ut_sb = [ap.tile([P, QT * P], BF16, tag=f"ut{i}",
                                     name=f"ut{i}") for i in range(DT)]
                    for dd in range(DT):
                        nc.sync.dma_start(out=wv_sb[dd][:],
                                          in_=wv[dd * P:(dd + 1) * P, :])

                    def scores_block(s0, s1, bext, q0):
                        e_tiles, e_offs, u_offs = [], [], []
                        for kt in range(bext):
                            # slots below ls_min never read k-tile kt
                            # (causal): narrow the moving dim, fp32r keeps
                            # N >= 256 (below which it slows 4x).
                            ls_min = kt // 2
                            offu = max(0, ls_min - s0) * P
                            offs = min(offu, 256)
                            n = 512 - offs
                            pscore = ps_s.tile([P, 512], F32, tag="sc")
                            for dd in range(DT):
                                nc.tensor.matmul(
                                    pscore[:, 0:n],
                                    xk_sb[dd][:, kt * P:(kt + 1) * P],
                                    qg_sb[dd][:, q0 + offs:q0 + 512],
                                    start=(dd == 0), stop=(dd == DT - 1),
                                )
                            et = ep.tile([P, 512], BF16, tag="E")
                            # E = exp(scores^T + w[k]) (w in the bias slot)
                            nc.scalar.activation(et[:, 0:n], pscore[:, 0:n],
                                                 EXP, bias=wb_sb[:, kt:kt + 1])
                            e_tiles.append(et)
                            e_offs.append(offs)
                            u_offs.append(offu)
                            # causal boundary mask on each slot's two
                            # diagonal k-tiles, applied eagerly so the U
                            # accumulation can batch whole q-blocks.
                            for ls in range(s0, s1):
                                if kt == EXT[ls] - 2 or kt == EXT[ls] - 1:
                                    j = kt - (EXT[ls] - 2)
                                    lo = (ls - s0) * P - offs
                                    nc.vector.tensor_mul(
                                        et[:, lo:lo + P],
                                        et[:, lo:lo + P],
                                        mask_sb[:, (2 * ls + j) * P:
                                                (2 * ls + j + 1) * P],
                                    )
                        return e_tiles, e_offs, u_offs

                    def u_block(bext, q0, e_tiles, e_offs, u_offs):
                        # U^T[d,q] = sum_k Xk[k,d]^T E[k,q].  Column ranges
                        # narrow monotonically with kt and nest inside the
                        # kt=0 full-width start=True write, so each column
                        # accumulates exactly its causal k-extent.
                        for dt in range(DT):
                            pu = ps_u.tile([P, 512], F32, tag="pu",
                                           name=f"pu{dt}")
                            for kt in range(bext):
                                ou, os_ = u_offs[kt], e_offs[kt]
                                nc.tensor.matmul(
                                    pu[:, ou:512],
                                    xkd_sb[kt][:, dt * P:(dt + 1) * P],
                                    e_tiles[kt][:, ou - os_:512 - os_],
                                    start=(kt == 0), stop=(kt == bext - 1),
                                )
                            nc.vector.tensor_copy(
                                ut_sb[dt][:, q0:q0 + 512], pu[:])

                    def rowsums_block(s0, s1, e_tiles, e_offs):
                        rcs = {}
                        for ls in range(s0, s1):
                            lq = (ls - s0) * P
                            pm = ps_o.tile([P, 1], F32, tag="po",
                                           name=f"pm{ls}")
                            for kt in range(EXT[ls]):
                                el = e_tiles[kt][:, lq - e_offs[kt]:
                                                 lq - e_offs[kt] + P]
                                nc.tensor.matmul(pm[:], el, ones_sb[:],
                                                 start=(kt == 0),
                                                 stop=(kt == EXT[ls] - 1))
                            rc = sp.tile([P, 1], F32, tag="rc",
                                         name=f"rc{ls}")
                            nc.vector.reciprocal(rc[:], pm[:])
                            rcs[ls] = rc
                        return rcs

                    def out_block(s0, s1, rcs):
                        # out[q,d] = (U @ Wv) / rowsum + bv
                        for ls in range(s0, s1):
                            ot = op.tile([P, D], F32, tag="ot")
                            for j in range(2):
                                po = ps_o.tile([P, 512], F32, tag="po",
                                               name=f"po{ls}_{j}")
                                for dt in range(DT):
                                    nc.tensor.matmul(
                                        po[:],
                                        ut_sb[dt][:, ls * P:(ls + 1) * P],
                                        wv_sb[dt][:, j * 512:(j + 1) * 512],
                                        start=(dt == 0), stop=(dt == DT - 1),
                                    )
                                nc.vector.tensor_scalar_mul(
                                    ot[:, j * 512:(j + 1) * 512], po[:],
                                    rcs[ls][:])
                                nc.vector.tensor_add(
                                    ot[:, j * 512:(j + 1) * 512],
                                    ot[:, j * 512:(j + 1) * 512],
                                    bv_sb[:, j * 512:(j + 1) * 512])
                                nc.sync.dma_start(
                                    out=y[ls * P:(ls + 1) * P,
                                          j * 512:(j + 1) * 512],
                                    in_=ot[:, j * 512:(j + 1) * 512])

                    s0, s1, bext = BLK[0]
                    e0, eo0, uo0 = scores_block(s0, s1, bext, s0 * P)
                    u_block(bext, s0 * P, e0, eo0, uo0)
                    rcs0 = rowsums_block(s0, s1, e0, eo0)

                    t0, t1, bext1 = BLK[1]
                    e1, eo1, uo1 = scores_block(t0, t1, bext1, t0 * P)
                    out_block(s0, s1, rcs0)
                    u_block(bext1, t0 * P, e1, eo1, uo1)
                    rcs1 = rowsums_block(t0, t1, e1, eo1)
                    out_block(t0, t1, rcs1)

    nc.compile()
    return nc


def _get_nc():
    if "nc" not in _CACHE:
        _CACHE["nc"] = _build()
    return _CACHE["nc"]


def make_in_maps(X, Wq, bq, Wk, bk, Wv, bv):
    X = np.asarray(X, np.float32)
    Wq = np.asarray(Wq, np.float32)
    Wk = np.asarray(Wk, np.float32)
    Wv = np.ascontiguousarray(np.asarray(Wv, np.float32))
    bq = np.asarray(bq, np.float32)
    bv = np.asarray(bv, np.float32)

    G = Wq @ Wk.T                                # [D, D]
    # do-major retiling: gg[p, do*1024 + dd*128 + c] = G[dd*128+p, do*128+c]
    gg = np.ascontiguousarray(
        G.reshape(DT, P, DT, P).transpose(1, 2, 0, 3).reshape(P, DT * D))
    wkbq = Wk @ bq                               # [D]
    bvp = np.ascontiguousarray(np.broadcast_to(bv[None, :], (P, D)))
    wv16 = Wv.astype(ml_dtypes.bfloat16)

    masks = {}
    for h in (0, 1):
        m = np.zeros((QT, 2 * P, P), np.float32)
        for s in range(QT):
            qt = QTS[h][s]
            kk = (2 * s) * P + np.arange(2 * P)[:, None]
            qq = qt * P + np.arange(P)[None, :]
            m[s] = (kk <= qq)
        # [s, kk, q] -> [kk%128, s*256 + (kk//128)*128 + q]
        m2 = m.reshape(QT, 2, P, P).transpose(2, 0, 1, 3).reshape(P, QT * 2 * P)
        masks[h] = np.ascontiguousarray(m2.astype(ml_dtypes.bfloat16))

    in_maps = []
    for c in range(8):
        b, h = divmod(c, 2)
        Xb = X[b]
        xkt = np.ascontiguousarray(Xb.T)
        xkd = np.ascontiguousarray(Xb.astype(ml_dtypes.bfloat16))
        xq_rows = np.concatenate(
            [Xb[qt * P:(qt + 1) * P] for qt in QTS[h]], axis=0)
        xqt = np.ascontiguousarray(xq_rows.T)
        w = Xb @ wkbq                             # [S] additive k-bias
        wbp = np.ascontiguousarray(w.reshape(KT, P).T)   # [P, KT]
        in_maps.append({
            "xqt": xqt, "xkt": xkt, "g": gg, "xkd": xkd, "wv": wv16,
            "wb": wbp, "bvp": bvp, "msk": masks[h],
        })
    return in_maps


def assemble(results):
    Y = np.empty((B, S, D), np.float32)
    for c in range(8):
        b, h = divmod(c, 2)
        yc = results[c]["y"]
        for s in range(QT):
            qt = QTS[h][s]
            Y[b, qt * P:(qt + 1) * P, :] = yc[s * P:(s + 1) * P, :]
    return Y


def kernel(X, Wq, bq, Wk, bk, Wv, bv):
    nc = _get_nc()
    in_maps = make_in_maps(X, Wq, bq, Wk, bk, Wv, bv)
    res = run_bass_kernel_spmd(nc, in_maps, core_ids=list(range(8)))
    return assemble(res.results)


# revision 14
# speedup vs baseline: 1.2340x; 1.0637x over previous
"""Causal single-head attention layer on 8 TRN2 NeuronCores.

Problem: X[4,2048,1024]; Q/K/V = X@W+b; scores = Q@K^T (no 1/sqrt(d));
causal mask; softmax; out = P@V.

Sharding: 2 cores per batch. Each core owns 8 query tiles (128 rows) of
its batch, folded for causal load balance:
  core h=0 -> global q-tiles (0,3,4,7,8,11,12,15)
  core h=1 -> global q-tiles (1,2,5,6,9,10,13,14)
Slot s on either core has causal extent <= 2s+2 k-tiles, so ONE uniform
program runs on all 8 cores; the exact causal boundary is a host-supplied
0/1 mask over the last two k-tiles of each slot.

Math restructuring:
  scores = (XqWq+bq)(XkWk+bk)^T
         = Xq G Xk^T + [q-only term] + w[k] + [const],  G = Wq Wk^T (host)
  q-only and const terms cancel in softmax; w[k] = Xk @ (Wk bq) (host)
  rides the per-partition bias slot of the Exp activation.
  The V projection is folded PAST the attention sum (associativity):
    out = (E @ Xk) @ Wv / rowsum + bv = U @ Wv / rowsum + bv
  so the [S,D]x[D,D] V projection (duplicated on both cores of a batch)
  is replaced by a per-core [1024,D]x[D,D] output projection.

On-device layout (contraction always on partitions):
  host passes X^T (xkt, f32r) for scores, X (xkd, bf16) for the U
  accumulation; scores^T[k,q] accumulate fp32 in PSUM; E = exp(scores^T
  + w[k]) in bf16 (w rides the Exp bias slot); U^T[d,q] = sum_k Xk^T E
  accumulates per d-tile in PSUM with causally narrowed moving dims
  (bf16 has no N>=256 restriction; fp32r scores clamp at N=256);
  row sums via matmul with ones; out[q,d] = (U@Wv)/sums + bv, Wv bf16.
  No max-subtraction needed: |scores| <= ~60 so exp stays in range.

Scheduling notes (PE pstate: any PE idle gap drops the clock for ~3us,
so the program is ordered to keep PE streaks long):
  - One flat PSUM pool set spans Qg + attention (Qg chains ride the U
    pool's ring) so there is no PSUM pool-transition barrier between
    the Qg copies and the first score matmul.
  - Qg runs as per-(do,c) chains, all c=0 chains first: their copies
    trail one chain behind, and scores of block 0 (which read only the
    c=0 half of Qg) start right after the c=1 chains with no wait.
  - DMA order = consumption order: (g|xq interleaved), xkt k-blocks
    0-1, wb+mask, xkd tiles 0-7, xkt k-blocks 2-3, xkd 8-15, wv, bvp.
  - PE order: Qg-c0, Qg-c1, [b0 scores, U, rowsums], [b1 scores],
    [b0 out], [b1 U, rowsums, out].
"""

import numpy as np
import ml_dtypes

import concourse.bass as bass  # noqa: F401
import concourse.mybir as mybir
from concourse import bacc
from concourse.bass_utils import run_bass_kernel_spmd
from concourse.tile import TileContext

F32 = mybir.dt.float32
F32R = mybir.dt.float32r
BF16 = mybir.dt.bfloat16
EXP = mybir.ActivationFunctionType.Exp

B, S, D = 4, 2048, 1024
P = 128
DT = D // P          # 8 d-tiles
QT = 8               # q-tile slots per core
KT = S // P          # 16 k-tiles
EXT = [2 * s + 2 for s in range(QT)]   # uniform per-slot k-extent
BLK = [(0, 4, 8), (4, 8, 16)]          # (slot_lo, slot_hi, block k-extent)

QTS = {0: [0, 3, 4, 7, 8, 11, 12, 15], 1: [1, 2, 5, 6, 9, 10, 13, 14]}

_CACHE = {}


def _build(reps=1):
    nc = bacc.Bacc("TRN2", target_bir_lowering=False, debug=False, num_devices=8)
    xqt = nc.declare_dram_parameter("xqt", [D, QT * P], F32R, isOutput=False)
    xkt = nc.declare_dram_parameter("xkt", [D, S], F32R, isOutput=False)
    g = nc.declare_dram_parameter("g", [P, DT * D], F32R, isOutput=False)
    xkd = nc.declare_dram_parameter("xkd", [S, D], BF16, isOutput=False)
    wv = nc.declare_dram_parameter("wv", [D, D], BF16, isOutput=False)
    wb = nc.declare_dram_parameter("wb", [P, KT], F32, isOutput=False)
    bvp = nc.declare_dram_parameter("bvp", [P, D], F32, isOutput=False)
    msk = nc.declare_dram_parameter("msk", [P, QT * 2 * P], BF16, isOutput=False)
    y = nc.declare_dram_parameter("y", [QT * P, D], F32, isOutput=True)

    with TileContext(nc) as tc:
      for _rep in range(reps):
        with tc.tile_pool(name="persist", bufs=1) as pp:
            # ---- persistent tiles ----
            xk_sb = [pp.tile([P, S], F32R, tag=f"xk{i}", name=f"xk{i}")
                     for i in range(DT)]
            qg_sb = [pp.tile([P, QT * P], F32R, tag=f"qg{i}", name=f"qg{i}")
                     for i in range(DT)]
            xkd_sb = [pp.tile([P, D], BF16, tag=f"xkd{i}", name=f"xkd{i}")
                      for i in range(KT)]
            wb_sb = pp.tile([P, KT], F32, tag="wb", name="wb_sb")
            bv_sb = pp.tile([P, D], F32, tag="bv", name="bv_sb")
            mask_sb = pp.tile([P, QT * 2 * P], BF16, tag="mask", name="mask_sb")
            ones_sb = pp.tile([P, 1], BF16, tag="ones", name="ones_sb")
            junk_sb = pp.tile([P, 512], BF16, tag="junk", name="junk_sb")

            with (
                tc.tile_pool(name="pssc", bufs=2, space="PSUM") as ps_s,
                tc.tile_pool(name="psu", bufs=4, space="PSUM") as ps_u,
                tc.tile_pool(name="pso", bufs=2, space="PSUM") as ps_o,
            ):
                # ---- Qg phase: Qg^T[d2,q] = sum_d1 G[d1,d2] Xq^T[d1,q] ----
                with tc.tile_pool(name="qgpool", bufs=1) as qp:
                    xq_sb = [qp.tile([P, QT * P], F32R, tag=f"xq{i}",
                                     name=f"xq{i}") for i in range(DT)]
                    # g_sb[do] cols dd*128+c hold G[dd*128+p, do*128+c]
                    # (host-retiled so each do-block is one large DMA).
                    g_sb = [qp.tile([P, D], F32R, tag=f"g{i}", name=f"g{i}")
                            for i in range(DT)]
                    # c=0 chains need only the first halves of xq, so land
                    # those, then the g blocks, then the second halves.
                    for dd in range(DT):
                        nc.sync.dma_start(out=xq_sb[dd][:, 0:512],
                                          in_=xqt[dd * P:(dd + 1) * P, 0:512])
                    for do in range(DT):
                        nc.sync.dma_start(out=g_sb[do][:],
                                          in_=g[:, do * D:(do + 1) * D])
                    for dd in range(DT):
                        nc.sync.dma_start(out=xq_sb[dd][:, 512:1024],
                                          in_=xqt[dd * P:(dd + 1) * P,
                                                  512:1024])
                    # attention inputs stream in behind the Qg inputs, in
                    # consumption order (k-block-major for the scores).
                    for kb in range(2):
                        for dd in range(DT):
                            nc.sync.dma_start(
                                out=xk_sb[dd][:, kb * 512:(kb + 1) * 512],
                                in_=xkt[dd * P:(dd + 1) * P,
                                        kb * 512:(kb + 1) * 512])
                    nc.sync.dma_start(out=wb_sb[:], in_=wb[:])
                    nc.sync.dma_start(out=mask_sb[:], in_=msk[:])
                    for kt in range(8):
                        nc.sync.dma_start(out=xkd_sb[kt][:],
                                          in_=xkd[kt * P:(kt + 1) * P, :])
                    for kb in range(2, 4):
                        for dd in range(DT):
                            nc.sync.dma_start(
                                out=xk_sb[dd][:, kb * 512:(kb + 1) * 512],
                                in_=xkt[dd * P:(dd + 1) * P,
                                        kb * 512:(kb + 1) * 512])
                    for kt in range(8, KT):
                        nc.sync.dma_start(out=xkd_sb[kt][:],
                                          in_=xkd[kt * P:(kt + 1) * P, :])
                    nc.sync.dma_start(out=bv_sb[:], in_=bvp[:])
                    nc.gpsimd.memset(ones_sb[:], 1.0)
                    nc.gpsimd.memset(junk_sb[:], 0.0)

                    # PE warm-up: dummy matmuls spanning the initial DMA
                    # wait (~10us) so the p-state ramp completes before the
                    # first real chain; the PE would otherwise idle here and
                    # every post-gap instruction pays the slow-clock ramp.
                    for w in range(38):
                        pw = ps_o.tile([1, 512], F32, tag="po",
                                       name=f"warm{w}")
                        nc.tensor.matmul(pw[:], ones_sb[:], junk_sb[:],
                                         start=True, stop=True)

                    # Qg chains ride the psu ring: no fresh PSUM pool, so
                    # the first score matmul has no pool-transition wait.
                    for c in range(2):
                        for do in range(DT):
                            pq = ps_u.tile([P, 512], F32, tag="pu",
                                           name=f"pq{c}_{do}")
                            for dd in range(DT):
                                nc.tensor.matmul(
                                    pq[:],
                                    g_sb[do][:, dd * P:(dd + 1) * P],
                                    xq_sb[dd][:, c * 512:(c + 1) * 512],
                                    start=(dd == 0), stop=(dd == DT - 1),
                                )
                            nc.vector.tensor_copy(
                                qg_sb[do][:, c * 512:(c + 1) * 512], pq[:])

                # ---- Attention ----
                with (
                    tc.tile_pool(name="attn", bufs=1) as ap,
                    tc.tile_pool(name="estage", bufs=24) as ep,
                    tc.tile_pool(name="ostage", bufs=2) as op,
                    tc.tile_pool(name="small", bufs=4) as sp,
                ):
                    wv_sb = [ap.tile([P, D], BF16, tag=f"wvx{i}",
                                     name=f"wvx{i}") for i in range(DT)]
                    ut_sb = [ap.tile([P, QT * P], BF16, tag=f"ut{i}",
                                     name=f"ut{i}") for i in range(DT)]
                    for dd in range(DT):
                        nc.sync.dma_start(out=wv_sb[dd][:],
                                          in_=wv[dd * P:(dd + 1) * P, :])

                    def scores_block(s0, s1, bext, q0):
                        e_tiles, e_offs, u_offs = [], [], []
                        for kt in range(bext):
                            # slots below ls_min never read k-tile kt
                            # (causal): narrow the moving dim, fp32r keeps
                            # N >= 256 (below which it slows 4x).
                            ls_min = kt // 2
                            offu = max(0, ls_min - s0) * P
                            offs = min(offu, 256)
                            n = 512 - offs
                            pscore = ps_s.tile([P, 512], F32, tag="sc")
                            for dd in range(DT):
                                nc.tensor.matmul(
                                    pscore[:, 0:n],
                                    xk_sb[dd][:, kt * P:(kt + 1) * P],
                                    qg_sb[dd][:, q0 + offs:q0 + 512],
                                    start=(dd == 0), stop=(dd == DT - 1),
                                )
                            et = ep.tile([P, 512], BF16, tag="E")
                            # E = exp(scores^T + w[k]) (w in the bias slot)
                            nc.scalar.activation(et[:, 0:n], pscore[:, 0:n],
                                                 EXP, bias=wb_sb[:, kt:kt + 1])
                            e_tiles.append(et)
                            e_offs.append(offs)
                            u_offs.append(offu)
                            # causal boundary mask on each slot's two
                            # diagonal k-tiles, applied eagerly so the U
                            # accumulation can batch whole q-blocks.
                            for ls in range(s0, s1):
                                if kt == EXT[ls] - 2 or kt == EXT[ls] - 1:
                                    j = kt - (EXT[ls] - 2)
                                    lo = (ls - s0) * P - offs
                                    nc.vector.tensor_mul(
                                        et[:, lo:lo + P],
                                        et[:, lo:lo + P],
                                        mask_sb[:, (2 * ls + j) * P:
                                                (2 * ls + j + 1) * P],
                                    )
                        return e_tiles, e_offs, u_offs

                    def u_block(bext, q0, e_tiles, e_offs, u_offs):
                        # U^T[d,q] = sum_k Xk[k,d]^T E[k,q].  Column ranges
                        # narrow monotonically with kt and nest inside the
                        # kt=0 full-width start=True write, so each column
                        # accumulates exactly its causal k-extent.
                        for dt in range(DT):
                            pu = ps_u.tile([P, 512], F32, tag="pu",
                                           name=f"pu{dt}")
                            for kt in range(bext):
                                ou, os_ = u_offs[kt], e_offs[kt]
                                nc.tensor.matmul(
                                    pu[:, ou:512],
                                    xkd_sb[kt][:, dt * P:(dt + 1) * P],
                                    e_tiles[kt][:, ou - os_:512 - os_],
                                    start=(kt == 0), stop=(kt == bext - 1),
                                )
                            # alternate copy engines so the trailing copies
                            # (which gate the out-projection) drain 2x fast
                            if dt % 2 == 0:
                                nc.vector.tensor_copy(
                                    ut_sb[dt][:, q0:q0 + 512], pu[:])
                            else:
                                nc.scalar.copy(
                                    ut_sb[dt][:, q0:q0 + 512], pu[:])

                    def rowsums_block(s0, s1, e_tiles, e_offs):
                        rcs = {}
                        for ls in range(s0, s1):
                            lq = (ls - s0) * P
                            pm = ps_o.tile([P, 1], F32, tag="po",
                                           name=f"pm{ls}")
                            for kt in range(EXT[ls]):
                                el = e_tiles[kt][:, lq - e_offs[kt]:
                                                 lq - e_offs[kt] + P]
                                nc.tensor.matmul(pm[:], el, ones_sb[:],
                                                 start=(kt == 0),
                                                 stop=(kt == EXT[ls] - 1))
                            rc = sp.tile([P, 1], F32, tag="rc",
                                         name=f"rc{ls}")
                            nc.vector.reciprocal(rc[:], pm[:])
                            rcs[ls] = rc
                        return rcs

                    def out_block(s0, s1, rcs, last=False):
                        # out[q,d] = (U @ Wv) / rowsum + bv
                        for ls in range(s0, s1):
                            ot = op.tile([P, D], F32, tag="ot")
                            pieces = [(0, 512), (512, 1024)]
                            if last and ls == s1 - 1:
                                # fine-grain the final tile so the trailing
                                # DVE scale + store DMA overlap the last
                                # matmul chains instead of serializing
                                pieces = [(0, 512), (512, 768),
                                          (768, 896), (896, 1024)]
                            for (lo, hi) in pieces:
                                po = ps_o.tile([P, 512], F32, tag="po",
                                               name=f"po{ls}_{lo}")
                                pv = po[:, 0:hi - lo]
                                for dt in range(DT):
                                    nc.tensor.matmul(
                                        pv,
                                        ut_sb[dt][:, ls * P:(ls + 1) * P],
                                        wv_sb[dt][:, lo:hi],
                                        start=(dt == 0), stop=(dt == DT - 1),
                                    )
                                nc.vector.tensor_scalar_mul(
                                    ot[:, lo:hi], pv, rcs[ls][:])
                                nc.vector.tensor_add(
                                    ot[:, lo:hi], ot[:, lo:hi],
                                    bv_sb[:, lo:hi])
                                nc.sync.dma_start(
                                    out=y[ls * P:(ls + 1) * P, lo:hi],
                                    in_=ot[:, lo:hi])

                    s0, s1, bext = BLK[0]
                    e0, eo0, uo0 = scores_block(s0, s1, bext, s0 * P)
                    u_block(bext, s0 * P, e0, eo0, uo0)
                    rcs0 = rowsums_block(s0, s1, e0, eo0)

                    t0, t1, bext1 = BLK[1]
                    e1, eo1, uo1 = scores_block(t0, t1, bext1, t0 * P)
                    out_block(s0, s1, rcs0)
                    u_block(bext1, t0 * P, e1, eo1, uo1)
                    rcs1 = rowsums_block(t0, t1, e1, eo1)
                    out_block(t0, t1, rcs1, last=True)

    nc.compile()
    return nc


def _get_nc():
    if "nc" not in _CACHE:
        _CACHE["nc"] = _build()
    return _CACHE["nc"]


def make_in_maps(X, Wq, bq, Wk, bk, Wv, bv):
    X = np.asarray(X, np.float32)
    Wq = np.asarray(Wq, np.float32)
    Wk = np.asarray(Wk, np.float32)
    Wv = np.ascontiguousarray(np.asarray(Wv, np.float32))
    bq = np.asarray(bq, np.float32)
    bv = np.asarray(bv, np.float32)

    G = Wq @ Wk.T                                # [D, D]
    # do-major retiling: gg[p, do*1024 + dd*128 + c] = G[dd*128+p, do*128+c]
    gg = np.ascontiguousarray(
        G.reshape(DT, P, DT, P).transpose(1, 2, 0, 3).reshape(P, DT * D))
    wkbq = Wk @ bq                               # [D]
    bvp = np.ascontiguousarray(np.broadcast_to(bv[None, :], (P, D)))
    wv16 = Wv.astype(ml_dtypes.bfloat16)

    masks = {}
    for h in (0, 1):
        m = np.zeros((QT, 2 * P, P), np.float32)
        for s in range(QT):
            qt = QTS[h][s]
            kk = (2 * s) * P + np.arange(2 * P)[:, None]
            qq = qt * P + np.arange(P)[None, :]
            m[s] = (kk <= qq)
        # [s, kk, q] -> [kk%128, s*256 + (kk//128)*128 + q]
        m2 = m.reshape(QT, 2, P, P).transpose(2, 0, 1, 3).reshape(P, QT * 2 * P)
        masks[h] = np.ascontiguousarray(m2.astype(ml_dtypes.bfloat16))

    in_maps = []
    for c in range(8):
        b, h = divmod(c, 2)
        Xb = X[b]
        xkt = np.ascontiguousarray(Xb.T)
        xkd = np.ascontiguousarray(Xb.astype(ml_dtypes.bfloat16))
        xq_rows = np.concatenate(
            [Xb[qt * P:(qt + 1) * P] for qt in QTS[h]], axis=0)
        xqt = np.ascontiguousarray(xq_rows.T)
        w = Xb @ wkbq                             # [S] additive k-bias
        wbp = np.ascontiguousarray(w.reshape(KT, P).T)   # [P, KT]
        in_maps.append({
            "xqt": xqt, "xkt": xkt, "g": gg, "xkd": xkd, "wv": wv16,
            "wb": wbp, "bvp": bvp, "msk": masks[h],
        })
    return in_maps


def assemble(results):
    Y = np.empty((B, S, D), np.float32)
    for c in range(8):
        b, h = divmod(c, 2)
        yc = results[c]["y"]
        for s in range(QT):
            qt = QTS[h][s]
            Y[b, qt * P:(qt + 1) * P, :] = yc[s * P:(s + 1) * P, :]
    return Y


def kernel(X, Wq, bq, Wk, bk, Wv, bv):
    nc = _get_nc()
    in_maps = make_in_maps(X, Wq, bq, Wk, bk, Wv, bv)
    res = run_bass_kernel_spmd(nc, in_maps, core_ids=list(range(8)))
    return assemble(res.results)


# revision 25
# speedup vs baseline: 1.2429x; 1.0072x over previous
"""Causal single-head attention layer on 8 TRN2 NeuronCores.

Problem: X[4,2048,1024]; Q/K/V = X@W+b; scores = Q@K^T (no 1/sqrt(d));
causal mask; softmax; out = P@V.

Sharding: 2 cores per batch. Each core owns 8 query tiles (128 rows) of
its batch, folded for causal load balance:
  core h=0 -> global q-tiles (0,3,4,7,8,11,12,15)
  core h=1 -> global q-tiles (1,2,5,6,9,10,13,14)
Slot s on either core has causal extent <= 2s+2 k-tiles, so ONE uniform
program runs on all 8 cores; the exact causal boundary is a host-supplied
0/1 mask over the last two k-tiles of each slot.

Math restructuring:
  scores = (XqWq+bq)(XkWk+bk)^T
         = Xq G Xk^T + [q-only term] + w[k] + [const],  G = Wq Wk^T (host)
  q-only and const terms cancel in softmax; w[k] = Xk @ (Wk bq) (host)
  rides the per-partition bias slot of the Exp activation.
  The V projection is folded PAST the attention sum (associativity):
    out = (E @ Xk) @ Wv / rowsum + bv = U @ Wv / rowsum + bv
  so the [S,D]x[D,D] V projection (duplicated on both cores of a batch)
  is replaced by a per-core [1024,D]x[D,D] output projection.

On-device layout (contraction always on partitions):
  host passes X^T (xkt, f32r) for scores, X (xkd, bf16) for the U
  accumulation; scores^T[k,q] accumulate fp32 in PSUM; E = exp(scores^T
  + w[k]) in bf16 (w rides the Exp bias slot); U^T[d,q] = sum_k Xk^T E
  accumulates per d-tile in PSUM with causally narrowed moving dims
  (bf16 has no N>=256 restriction; fp32r scores clamp at N=256);
  row sums via matmul with ones; out[q,d] = (U@Wv)/sums + bv, Wv bf16.
  No max-subtraction needed: |scores| <= ~60 so exp stays in range.

Scheduling notes (PE pstate: any PE idle gap drops the clock for ~3us,
so the program is ordered to keep PE streaks long):
  - One flat PSUM pool set spans Qg + attention (Qg chains ride the U
    pool's ring) so there is no PSUM pool-transition barrier between
    the Qg copies and the first score matmul.
  - Qg runs as per-(do,c) chains, all c=0 chains first: their copies
    trail one chain behind, and scores of block 0 (which read only the
    c=0 half of Qg) start right after the c=1 chains with no wait.
  - DMA order = consumption order: (g|xq interleaved), xkt k-blocks
    0-1, wb+mask, xkd tiles 0-7, xkt k-blocks 2-3, xkd 8-15, wv, bvp.
  - PE order: Qg-c0, Qg-c1, [b0 scores, U, rowsums], [b1 scores],
    [b0 out], [b1 U, rowsums, out].
"""

import numpy as np
import ml_dtypes

import concourse.bass as bass  # noqa: F401
import concourse.mybir as mybir
from concourse import bacc
from concourse.bass_utils import run_bass_kernel_spmd
from concourse.tile import TileContext

F32 = mybir.dt.float32
F32R = mybir.dt.float32r
BF16 = mybir.dt.bfloat16
EXP = mybir.ActivationFunctionType.Exp
MULT = mybir.AluOpType.mult
ADD = mybir.AluOpType.add

B, S, D = 4, 2048, 1024
P = 128
DT = D // P          # 8 d-tiles
QT = 8               # q-tile slots per core
KT = S // P          # 16 k-tiles
EXT = [2 * s + 2 for s in range(QT)]   # uniform per-slot k-extent
BLK = [(0, 4, 8), (4, 8, 16)]          # (slot_lo, slot_hi, block k-extent)

QTS = {0: [0, 3, 4, 7, 8, 11, 12, 15], 1: [1, 2, 5, 6, 9, 10, 13, 14]}

_CACHE = {}


def _build(reps=1):
    nc = bacc.Bacc("TRN2", target_bir_lowering=False, debug=False, num_devices=8)
    xqt = nc.declare_dram_parameter("xqt", [D, QT * P], F32R, isOutput=False)
    xkt = nc.declare_dram_parameter("xkt", [D, S], F32R, isOutput=False)
    g = nc.declare_dram_parameter("g", [P, DT * D], F32R, isOutput=False)
    xkd = nc.declare_dram_parameter("xkd", [S, D], BF16, isOutput=False)
    wv = nc.declare_dram_parameter("wv", [D, D], BF16, isOutput=False)
    wb = nc.declare_dram_parameter("wb", [P, KT], F32, isOutput=False)
    bvp = nc.declare_dram_parameter("bvp", [P, D], F32, isOutput=False)
    msk = nc.declare_dram_parameter("msk", [P, QT * 2 * P], BF16, isOutput=False)
    y = nc.declare_dram_parameter("y", [QT * P, D], F32, isOutput=True)

    with TileContext(nc) as tc:
      for _rep in range(reps):
        with tc.tile_pool(name="persist", bufs=1) as pp:
            # ---- persistent tiles ----
            xk_sb = [pp.tile([P, S], F32R, tag=f"xk{i}", name=f"xk{i}")
                     for i in range(DT)]
            qg_sb = [pp.tile([P, QT * P], F32R, tag=f"qg{i}", name=f"qg{i}")
                     for i in range(DT)]
            xkd_sb = [pp.tile([P, D], BF16, tag=f"xkd{i}", name=f"xkd{i}")
                      for i in range(KT)]
            wb_sb = pp.tile([P, KT], F32, tag="wb", name="wb_sb")
            bv_sb = pp.tile([P, D], F32, tag="bv", name="bv_sb")
            mask_sb = pp.tile([P, QT * 2 * P], BF16, tag="mask", name="mask_sb")
            ones_sb = pp.tile([P, 1], BF16, tag="ones", name="ones_sb")
            junk_sb = pp.tile([P, 512], BF16, tag="junk", name="junk_sb")

            with (
                tc.tile_pool(name="pssc", bufs=2, space="PSUM") as ps_s,
                tc.tile_pool(name="psu", bufs=4, space="PSUM") as ps_u,
                tc.tile_pool(name="pso", bufs=2, space="PSUM") as ps_o,
            ):
                # ---- Qg phase: Qg^T[d2,q] = sum_d1 G[d1,d2] Xq^T[d1,q] ----
                with tc.tile_pool(name="qgpool", bufs=1) as qp:
                    xq_sb = [qp.tile([P, QT * P], F32R, tag=f"xq{i}",
                                     name=f"xq{i}") for i in range(DT)]
                    # g_sb[do] cols dd*128+c hold G[dd*128+p, do*128+c]
                    # (host-retiled so each do-block is one large DMA).
                    g_sb = [qp.tile([P, D], F32R, tag=f"g{i}", name=f"g{i}")
                            for i in range(DT)]
                    # c=0 chains need only the first halves of xq, so land
                    # those, then the g blocks, then the second halves.
                    for dd in range(DT):
                        nc.sync.dma_start(out=xq_sb[dd][:, 0:512],
                                          in_=xqt[dd * P:(dd + 1) * P, 0:512])
                    for do in range(DT):
                        nc.sync.dma_start(out=g_sb[do][:],
                                          in_=g[:, do * D:(do + 1) * D])
                    for dd in range(DT):
                        nc.sync.dma_start(out=xq_sb[dd][:, 512:1024],
                                          in_=xqt[dd * P:(dd + 1) * P,
                                                  512:1024])
                    # attention inputs stream in behind the Qg inputs, in
                    # consumption order (k-block-major for the scores).
                    for kb in range(2):
                        for dd in range(DT):
                            nc.sync.dma_start(
                                out=xk_sb[dd][:, kb * 512:(kb + 1) * 512],
                                in_=xkt[dd * P:(dd + 1) * P,
                                        kb * 512:(kb + 1) * 512])
                    nc.sync.dma_start(out=wb_sb[:], in_=wb[:])
                    nc.sync.dma_start(out=mask_sb[:], in_=msk[:])
                    for kt in range(8):
                        nc.sync.dma_start(out=xkd_sb[kt][:],
                                          in_=xkd[kt * P:(kt + 1) * P, :])
                    for kb in range(2, 4):
                        for dd in range(DT):
                            nc.sync.dma_start(
                                out=xk_sb[dd][:, kb * 512:(kb + 1) * 512],
                                in_=xkt[dd * P:(dd + 1) * P,
                                        kb * 512:(kb + 1) * 512])
                    for kt in range(8, KT):
                        nc.sync.dma_start(out=xkd_sb[kt][:],
                                          in_=xkd[kt * P:(kt + 1) * P, :])
                    nc.sync.dma_start(out=bv_sb[:], in_=bvp[:])
                    nc.gpsimd.memset(ones_sb[:], 1.0)
                    nc.gpsimd.memset(junk_sb[:], 0.0)

                    # PE warm-up: dummy matmuls spanning the initial DMA
                    # wait (~10us) so the p-state ramp completes before the
                    # first real chain; the PE would otherwise idle here and
                    # every post-gap instruction pays the slow-clock ramp.
                    for w in range(38):
                        pw = ps_o.tile([1, 512], F32, tag="po",
                                       name=f"warm{w}")
                        nc.tensor.matmul(pw[:], ones_sb[:], junk_sb[:],
                                         start=True, stop=True)

                    # Qg chains ride the psu ring: no fresh PSUM pool, so
                    # the first score matmul has no pool-transition wait.
                    for c in range(2):
                        for do in range(DT):
                            pq = ps_u.tile([P, 512], F32, tag="pu",
                                           name=f"pq{c}_{do}")
                            for dd in range(DT):
                                nc.tensor.matmul(
                                    pq[:],
                                    g_sb[do][:, dd * P:(dd + 1) * P],
                                    xq_sb[dd][:, c * 512:(c + 1) * 512],
                                    start=(dd == 0), stop=(dd == DT - 1),
                                )
                            nc.vector.tensor_copy(
                                qg_sb[do][:, c * 512:(c + 1) * 512], pq[:])

                # ---- Attention ----
                with (
                    tc.tile_pool(name="attn", bufs=1) as ap,
                    tc.tile_pool(name="estage", bufs=24) as ep,
                    tc.tile_pool(name="ostage", bufs=2) as op,
                    tc.tile_pool(name="small", bufs=4) as sp,
                ):
                    wv_sb = [ap.tile([P, D], BF16, tag=f"wvx{i}",
                                     name=f"wvx{i}") for i in range(DT)]
                    ut_sb = [ap.tile([P, QT * P], BF16, tag=f"ut{i}",
                                     name=f"ut{i}") for i in range(DT)]
                    for dd in range(DT):
                        nc.sync.dma_start(out=wv_sb[dd][:],
                                          in_=wv[dd * P:(dd + 1) * P, :])

                    def scores_block(s0, s1, bext, q0):
                        e_tiles, e_offs, u_offs = [], [], []
                        for kt in range(bext):
                            # slots below ls_min never read k-tile kt
                            # (causal): narrow the moving dim, fp32r keeps
                            # N >= 256 (below which it slows 4x).
                            ls_min = kt // 2
                            offu = max(0, ls_min - s0) * P
                            offs = min(offu, 256)
                            n = 512 - offs
                            pscore = ps_s.tile([P, 512], F32, tag="sc")
                            for dd in range(DT):
                                nc.tensor.matmul(
                                    pscore[:, 0:n],
                                    xk_sb[dd][:, kt * P:(kt + 1) * P],
                                    qg_sb[dd][:, q0 + offs:q0 + 512],
                                    start=(dd == 0), stop=(dd == DT - 1),
                                )
                            et = ep.tile([P, 512], BF16, tag="E")
                            # E = exp(scores^T + w[k]) (w in the bias slot)
                            nc.scalar.activation(et[:, 0:n], pscore[:, 0:n],
                                                 EXP, bias=wb_sb[:, kt:kt + 1])
                            e_tiles.append(et)
                            e_offs.append(offs)
                            u_offs.append(offu)
                            # causal boundary mask on each slot's two
                            # diagonal k-tiles, applied eagerly so the U
                            # accumulation can batch whole q-blocks.
                            for ls in range(s0, s1):
                                if kt == EXT[ls] - 2 or kt == EXT[ls] - 1:
                                    j = kt - (EXT[ls] - 2)
                                    lo = (ls - s0) * P - offs
                                    nc.vector.tensor_mul(
                                        et[:, lo:lo + P],
                                        et[:, lo:lo + P],
                                        mask_sb[:, (2 * ls + j) * P:
                                                (2 * ls + j + 1) * P],
                                    )
                        return e_tiles, e_offs, u_offs

                    def u_block(bext, q0, e_tiles, e_offs, u_offs):
                        # U^T[d,q] = sum_k Xk[k,d]^T E[k,q].  Column ranges
                        # narrow monotonically with kt and nest inside the
                        # kt=0 full-width start=True write, so each column
                        # accumulates exactly its causal k-extent.
                        for dt in range(DT):
                            pu = ps_u.tile([P, 512], F32, tag="pu",
                                           name=f"pu{dt}")
                            for kt in range(bext):
                                ou, os_ = u_offs[kt], e_offs[kt]
                                nc.tensor.matmul(
                                    pu[:, ou:512],
                                    xkd_sb[kt][:, dt * P:(dt + 1) * P],
                                    e_tiles[kt][:, ou - os_:512 - os_],
                                    start=(kt == 0), stop=(kt == bext - 1),
                                )
                            # alternate copy engines so the trailing copies
                            # (which gate the out-projection) drain 2x fast
                            if dt % 2 == 0:
                                nc.vector.tensor_copy(
                                    ut_sb[dt][:, q0:q0 + 512], pu[:])
                            else:
                                nc.scalar.copy(
                                    ut_sb[dt][:, q0:q0 + 512], pu[:])

                    def rowsums_block(s0, s1, e_tiles, e_offs):
                        rcs = {}
                        for ls in range(s0, s1):
                            lq = (ls - s0) * P
                            pm = ps_o.tile([P, 1], F32, tag="po",
                                           name=f"pm{ls}")
                            for kt in range(EXT[ls]):
                                el = e_tiles[kt][:, lq - e_offs[kt]:
                                                 lq - e_offs[kt] + P]
                                nc.tensor.matmul(pm[:], el, ones_sb[:],
                                                 start=(kt == 0),
                                                 stop=(kt == EXT[ls] - 1))
                            rc = sp.tile([P, 1], F32, tag="rc",
                                         name=f"rc{ls}")
                            nc.vector.reciprocal(rc[:], pm[:])
                            rcs[ls] = rc
                        return rcs

                    def out_block(s0, s1, rcs, last=False):
                        # out[q,d] = (U @ Wv) / rowsum + bv
                        for ls in range(s0, s1):
                            ot = op.tile([P, D], F32, tag="ot")
                            pieces = [(0, 512), (512, 1024)]
                            if last and ls == s1 - 1:
                                # fine-grain the final tile so the trailing
                                # DVE scale + store DMA overlap the last
                                # matmul chains instead of serializing
                                pieces = [(0, 512), (512, 768),
                                          (768, 896), (896, 1024)]
                            for (lo, hi) in pieces:
                                po = ps_o.tile([P, 512], F32, tag="po",
                                               name=f"po{ls}_{lo}")
                                pv = po[:, 0:hi - lo]
                                for dt in range(DT):
                                    nc.tensor.matmul(
                                        pv,
                                        ut_sb[dt][:, ls * P:(ls + 1) * P],
                                        wv_sb[dt][:, lo:hi],
                                        start=(dt == 0), stop=(dt == DT - 1),
                                    )
                                nc.vector.scalar_tensor_tensor(
                                    ot[:, lo:hi], pv, rcs[ls][:],
                                    bv_sb[:, lo:hi], op0=MULT, op1=ADD)
                                nc.sync.dma_start(
                                    out=y[ls * P:(ls + 1) * P, lo:hi],
                                    in_=ot[:, lo:hi])

                    s0, s1, bext = BLK[0]
                    e0, eo0, uo0 = scores_block(s0, s1, bext, s0 * P)
                    u_block(bext, s0 * P, e0, eo0, uo0)
                    rcs0 = rowsums_block(s0, s1, e0, eo0)

                    t0, t1, bext1 = BLK[1]
                    e1, eo1, uo1 = scores_block(t0, t1, bext1, t0 * P)
                    out_block(s0, s1, rcs0)
                    u_block(bext1, t0 * P, e1, eo1, uo1)
                    rcs1 = rowsums_block(t0, t1, e1, eo1)
                    out_block(t0, t1, rcs1, last=True)

    nc.compile()
    return nc


def _get_nc():
    if "nc" not in _CACHE:
        _CACHE["nc"] = _build()
    return _CACHE["nc"]


def make_in_maps(X, Wq, bq, Wk, bk, Wv, bv):
    X = np.asarray(X, np.float32)
    Wq = np.asarray(Wq, np.float32)
    Wk = np.asarray(Wk, np.float32)
    Wv = np.ascontiguousarray(np.asarray(Wv, np.float32))
    bq = np.asarray(bq, np.float32)
    bv = np.asarray(bv, np.float32)

    G = Wq @ Wk.T                                # [D, D]
    # do-major retiling: gg[p, do*1024 + dd*128 + c] = G[dd*128+p, do*128+c]
    gg = np.ascontiguousarray(
        G.reshape(DT, P, DT, P).transpose(1, 2, 0, 3).reshape(P, DT * D))
    wkbq = Wk @ bq                               # [D]
    bvp = np.ascontiguousarray(np.broadcast_to(bv[None, :], (P, D)))
    wv16 = Wv.astype(ml_dtypes.bfloat16)

    masks = {}
    for h in (0, 1):
        m = np.zeros((QT, 2 * P, P), np.float32)
        for s in range(QT):
            qt = QTS[h][s]
            kk = (2 * s) * P + np.arange(2 * P)[:, None]
            qq = qt * P + np.arange(P)[None, :]
            m[s] = (kk <= qq)
        # [s, kk, q] -> [kk%128, s*256 + (kk//128)*128 + q]
        m2 = m.reshape(QT, 2, P, P).transpose(2, 0, 1, 3).reshape(P, QT * 2 * P)
        masks[h] = np.ascontiguousarray(m2.astype(ml_dtypes.bfloat16))

    in_maps = []
    for c in range(8):
        b, h = divmod(c, 2)
        Xb = X[b]
        xkt = np.ascontiguousarray(Xb.T)
        xkd = np.ascontiguousarray(Xb.astype(ml_dtypes.bfloat16))
        xq_rows = np.concatenate(
            [Xb[qt * P:(qt + 1) * P] for qt in QTS[h]], axis=0)
        xqt = np.ascontiguousarray(xq_rows.T)
        w = Xb @ wkbq                             # [S] additive k-bias
        wbp = np.ascontiguousarray(w.reshape(KT, P).T)   # [P, KT]
        in_maps.append({
            "xqt": xqt, "xkt": xkt, "g": gg, "xkd": xkd, "wv": wv16,
            "wb": wbp, "bvp": bvp, "msk": masks[h],
        })
    return in_maps


def assemble(results):
    Y = np.empty((B, S, D), np.float32)
    for c in range(8):
        b, h = divmod(c, 2)
        yc = results[c]["y"]
        for s in range(QT):
            qt = QTS[h][s]
            Y[b, qt * P:(qt + 1) * P, :] = yc[s * P:(s + 1) * P, :]
    return Y


def kernel(X, Wq, bq, Wk, bk, Wv, bv):
    nc = _get_nc()
    in_maps = make_in_maps(X, Wq, bq, Wk, bk, Wv, bv)
    res = run_bass_kernel_spmd(nc, in_maps, core_ids=list(range(8)))
    return assemble(res.results)


# revision 35
# speedup vs baseline: 1.2824x; 1.0318x over previous
"""Causal single-head attention layer on 8 TRN2 NeuronCores.

Problem: X[4,2048,1024]; Q/K/V = X@W+b; scores = Q@K^T (no 1/sqrt(d));
causal mask; softmax; out = P@V.

Sharding: 2 cores per batch. Each core owns 8 query tiles (128 rows) of
its batch, folded for causal load balance:
  core h=0 -> global q-tiles (0,3,4,7,8,11,12,15)
  core h=1 -> global q-tiles (1,2,5,6,9,10,13,14)
Slot s on either core has causal extent <= 2s+2 k-tiles, so ONE uniform
program runs on all 8 cores; the exact causal boundary is a host-supplied
0/1 mask over the last two k-tiles of each slot.

Math restructuring:
  scores = (XqWq+bq)(XkWk+bk)^T
         = Xq G Xk^T + [q-only term] + w[k] + [const],  G = Wq Wk^T (host)
  q-only and const terms cancel in softmax; w[k] = Xk @ (Wk bq) (host)
  rides the per-partition bias slot of the Exp activation.
  The V projection is folded PAST the attention sum (associativity):
    out = (E @ Xk) @ Wv / rowsum + bv = U @ Wv / rowsum + bv
  so the [S,D]x[D,D] V projection (duplicated on both cores of a batch)
  is replaced by a per-core [1024,D]x[D,D] output projection.

On-device layout (contraction always on partitions):
  host passes X^T (xkt, f32r) for scores, X (xkd, bf16) for the U
  accumulation; scores^T[k,q] accumulate fp32 in PSUM; E = exp(scores^T
  + w[k]) in bf16 (w rides the Exp bias slot); U^T[d,q] = sum_k Xk^T E
  accumulates per d-tile in PSUM with causally narrowed moving dims
  (bf16 has no N>=256 restriction; fp32r scores clamp at N=256);
  row sums via matmul with ones; out[q,d] = (U@Wv)/sums + bv, Wv bf16.
  No max-subtraction needed: |scores| <= ~60 so exp stays in range.

Scheduling notes (PE pstate: any PE idle gap drops the clock for ~3us,
so the program is ordered to keep PE streaks long):
  - One flat PSUM pool set spans Qg + attention (Qg chains ride the U
    pool's ring) so there is no PSUM pool-transition barrier between
    the Qg copies and the first score matmul.
  - Qg runs as per-(do,c) chains, all c=0 chains first: their copies
    trail one chain behind, and scores of block 0 (which read only the
    c=0 half of Qg) start right after the c=1 chains with no wait.
  - DMA order = consumption order: (g|xq interleaved), xkt k-blocks
    0-1, wb+mask, xkd tiles 0-7, xkt k-blocks 2-3, xkd 8-15, wv, bvp.
  - PE order: Qg-c0, Qg-c1, [b0 scores, U, rowsums], [b1 scores],
    [b0 out], [b1 U, rowsums, out].
"""

import numpy as np
import ml_dtypes

import concourse.bass as bass  # noqa: F401
import concourse.mybir as mybir
from concourse import bacc
from concourse.bass_utils import run_bass_kernel_spmd
from concourse.tile import TileContext

F32 = mybir.dt.float32
F32R = mybir.dt.float32r
BF16 = mybir.dt.bfloat16
FP8 = mybir.dt.float8e4
EXP = mybir.ActivationFunctionType.Exp
MULT = mybir.AluOpType.mult
ADD = mybir.AluOpType.add
SUB = mybir.AluOpType.subtract
DROW = mybir.MatmulPerfMode.DoubleRow

B, S, D = 4, 2048, 1024
P = 128
DT = D // P          # 8 d-tiles
QT = 8               # q-tile slots per core
KT = S // P          # 16 k-tiles
EXT = [2 * s + 2 for s in range(QT)]   # uniform per-slot k-extent
BLK = [(0, 4, 8), (4, 8, 16)]          # (slot_lo, slot_hi, block k-extent)

QTS = {0: [0, 3, 4, 7, 8, 11, 12, 15], 1: [1, 2, 5, 6, 9, 10, 13, 14]}

_CACHE = {}


def _build(reps=1):
    nc = bacc.Bacc("TRN2", target_bir_lowering=False, debug=False, num_devices=8)
    xqt = nc.declare_dram_parameter("xqt", [D, QT * P], F32R, isOutput=False)
    xkt = nc.declare_dram_parameter("xkt", [D, S], F32R, isOutput=False)
    g = nc.declare_dram_parameter("g", [P, DT * D], F32R, isOutput=False)
    xkd = nc.declare_dram_parameter("xkd", [S, D], BF16, isOutput=False)
    wv8a = nc.declare_dram_parameter("wv8a", [(DT // 2) * P, 2 * D], FP8,
                                     isOutput=False)
    wv8b = nc.declare_dram_parameter("wv8b", [(DT // 2) * P, 2 * D], FP8,
                                     isOutput=False)
    ident = nc.declare_dram_parameter("ident", [P, P], F32, isOutput=False)
    wb = nc.declare_dram_parameter("wb", [P, KT], F32, isOutput=False)
    bvp = nc.declare_dram_parameter("bvp", [P, D], F32, isOutput=False)
    msk = nc.declare_dram_parameter("msk", [P, QT * 2 * P], BF16, isOutput=False)
    y = nc.declare_dram_parameter("y", [QT * P, D], F32, isOutput=True)

    with TileContext(nc) as tc:
      for _rep in range(reps):
        with tc.tile_pool(name="persist", bufs=1) as pp:
            # ---- persistent tiles ----
            xk_sb = [pp.tile([P, S], F32R, tag=f"xk{i}", name=f"xk{i}")
                     for i in range(DT)]
            qg_sb = [pp.tile([P, QT * P], F32R, tag=f"qg{i}", name=f"qg{i}")
                     for i in range(DT)]
            xkd_sb = [pp.tile([P, D], BF16, tag=f"xkd{i}", name=f"xkd{i}")
                      for i in range(KT)]
            wb_sb = pp.tile([P, KT], F32, tag="wb", name="wb_sb")
            bv_sb = pp.tile([P, D], F32, tag="bv", name="bv_sb")
            mask_sb = pp.tile([P, QT * 2 * P], BF16, tag="mask", name="mask_sb")
            ones_sb = pp.tile([P, 1], BF16, tag="ones", name="ones_sb")
            junk_sb = pp.tile([P, 512], BF16, tag="junk", name="junk_sb")
            id_sb = pp.tile([P, P], F32, tag="ident", name="id_sb")

            with (
                tc.tile_pool(name="pssc", bufs=2, space="PSUM") as ps_s,
                tc.tile_pool(name="psu", bufs=4, space="PSUM") as ps_u,
                tc.tile_pool(name="pso", bufs=2, space="PSUM") as ps_o,
            ):
                # ---- Qg phase: Qg^T[d2,q] = sum_d1 G[d1,d2] Xq^T[d1,q] ----
                with tc.tile_pool(name="qgpool", bufs=1) as qp:
                    xq_sb = [qp.tile([P, QT * P], F32R, tag=f"xq{i}",
                                     name=f"xq{i}") for i in range(DT)]
                    # g_sb[do] cols dd*128+c hold G[dd*128+p, do*128+c]
                    # (host-retiled so each do-block is one large DMA).
                    g_sb = [qp.tile([P, D], F32R, tag=f"g{i}", name=f"g{i}")
                            for i in range(DT)]
                    # c=0 chains need only the first halves of xq, so land
                    # those, then the g blocks, then the second halves.
                    for dd in range(DT):
                        nc.sync.dma_start(out=xq_sb[dd][:, 0:512],
                                          in_=xqt[dd * P:(dd + 1) * P, 0:512])
                    for do in range(DT):
                        nc.sync.dma_start(out=g_sb[do][:],
                                          in_=g[:, do * D:(do + 1) * D])
                    for dd in range(DT):
                        nc.sync.dma_start(out=xq_sb[dd][:, 512:1024],
                                          in_=xqt[dd * P:(dd + 1) * P,
                                                  512:1024])
                    # attention inputs stream in behind the Qg inputs, in
                    # consumption order (k-block-major for the scores).
                    for kb in range(2):
                        for dd in range(DT):
                            nc.sync.dma_start(
                                out=xk_sb[dd][:, kb * 512:(kb + 1) * 512],
                                in_=xkt[dd * P:(dd + 1) * P,
                                        kb * 512:(kb + 1) * 512])
                    nc.sync.dma_start(out=wb_sb[:], in_=wb[:])
                    nc.sync.dma_start(out=mask_sb[:], in_=msk[:])
                    nc.sync.dma_start(out=id_sb[:], in_=ident[:])
                    for kt in range(8):
                        nc.sync.dma_start(out=xkd_sb[kt][:],
                                          in_=xkd[kt * P:(kt + 1) * P, :])
                    for kb in range(2, 4):
                        for dd in range(DT):
                            nc.sync.dma_start(
                                out=xk_sb[dd][:, kb * 512:(kb + 1) * 512],
                                in_=xkt[dd * P:(dd + 1) * P,
                                        kb * 512:(kb + 1) * 512])
                    for kt in range(8, KT):
                        nc.sync.dma_start(out=xkd_sb[kt][:],
                                          in_=xkd[kt * P:(kt + 1) * P, :])
                    nc.sync.dma_start(out=bv_sb[:], in_=bvp[:])
                    nc.gpsimd.memset(ones_sb[:], 1.0)
                    nc.gpsimd.memset(junk_sb[:], 0.0)

                    # PE warm-up: dummy matmuls spanning the initial DMA
                    # wait (~10us) so the p-state ramp completes before the
                    # first real chain; the PE would otherwise idle here and
                    # every post-gap instruction pays the slow-clock ramp.
                    for w in range(38):
                        pw = ps_o.tile([1, 512], F32, tag="po",
                                       name=f"warm{w}")
                        nc.tensor.matmul(pw[:], ones_sb[:], junk_sb[:],
                                         start=True, stop=True)

                    # Qg chains ride the psu ring: no fresh PSUM pool, so
                    # the first score matmul has no pool-transition wait.
                    for c in range(2):
                        for do in range(DT):
                            pq = ps_u.tile([P, 512], F32, tag="pu",
                                           name=f"pq{c}_{do}")
                            for dd in range(DT):
                                nc.tensor.matmul(
                                    pq[:],
                                    g_sb[do][:, dd * P:(dd + 1) * P],
                                    xq_sb[dd][:, c * 512:(c + 1) * 512],
                                    start=(dd == 0), stop=(dd == DT - 1),
                                )
                            nc.vector.tensor_copy(
                                qg_sb[do][:, c * 512:(c + 1) * 512], pq[:])

                # ---- Attention ----
                with (
                    tc.tile_pool(name="attn", bufs=1) as ap,
                    tc.tile_pool(name="estage", bufs=18) as ep,
                    tc.tile_pool(name="ostage", bufs=2) as op,
                    tc.tile_pool(name="small", bufs=4) as sp,
                ):
                    # fp8 pair tiles for the DoubleRow out-projection with
                    # residual compensation: Ua/Wa are fp8 roundings of
                    # U/rowsum and 128*Wv; Ub/Wb their fp8 residuals.
                    # out = Ua@Wa + Ub@Wa + Ua@Wb (UbWb ~ 0.1%, dropped).
                    wv8a_sb = [ap.tile([P, 2, D], FP8, tag=f"wva{i}",
                                       name=f"wva{i}") for i in range(DT // 2)]
                    wv8b_sb = [ap.tile([P, 2, D], FP8, tag=f"wvb{i}",
                                       name=f"wvb{i}") for i in range(DT // 2)]
                    ut8a_sb = [ap.tile([P, 2, QT * P], FP8, tag=f"uta{i}",
                                       name=f"uta{i}") for i in range(DT // 2)]
                    ut8b_sb = [ap.tile([P, 2, QT * P], FP8, tag=f"utb{i}",
                                       name=f"utb{i}") for i in range(DT // 2)]
                    for t in range(DT // 2):
                        nc.sync.dma_start(out=wv8a_sb[t][:],
                                          in_=wv8a[t * P:(t + 1) * P, :])
                    for t in range(DT // 2):
                        nc.sync.dma_start(out=wv8b_sb[t][:],
                                          in_=wv8b[t * P:(t + 1) * P, :])

                    def scores_block(s0, s1, bext, q0):
                        e_tiles, e_offs, u_offs = [], [], []
                        for kt in range(bext):
                            # slots below ls_min never read k-tile kt
                            # (causal): narrow the moving dim, fp32r keeps
                            # N >= 256 (below which it slows 4x).
                            ls_min = kt // 2
                            offu = max(0, ls_min - s0) * P
                            offs = min(offu, 256)
                            n = 512 - offs
                            pscore = ps_s.tile([P, 512], F32, tag="sc")
                            for dd in range(DT):
                                nc.tensor.matmul(
                                    pscore[:, 0:n],
                                    xk_sb[dd][:, kt * P:(kt + 1) * P],
                                    qg_sb[dd][:, q0 + offs:q0 + 512],
                                    start=(dd == 0), stop=(dd == DT - 1),
                                )
                            et = ep.tile([P, 512], BF16, tag="E")
                            # E = exp(scores^T + w[k]) (w in the bias slot)
                            nc.scalar.activation(et[:, 0:n], pscore[:, 0:n],
                                                 EXP, bias=wb_sb[:, kt:kt + 1])
                            e_tiles.append(et)
                            e_offs.append(offs)
                            u_offs.append(offu)
                            # causal boundary mask on each slot's two
                            # diagonal k-tiles, applied eagerly so the U
                            # accumulation can batch whole q-blocks.
                            for ls in range(s0, s1):
                                if kt == EXT[ls] - 2 or kt == EXT[ls] - 1:
                                    j = kt - (EXT[ls] - 2)
                                    lo = (ls - s0) * P - offs
                                    nc.vector.tensor_mul(
                                        et[:, lo:lo + P],
                                        et[:, lo:lo + P],
                                        mask_sb[:, (2 * ls + j) * P:
                                                (2 * ls + j + 1) * P],
                                    )
                        return e_tiles, e_offs, u_offs

                    def u_block(bext, q0, e_tiles, e_offs, u_offs, rcb):
                        # U^T[d,q] = sum_k Xk[k,d]^T E[k,q].  Column ranges
                        # narrow monotonically with kt and nest inside the
                        # kt=0 full-width start=True write, so each column
                        # accumulates exactly its causal k-extent.  The
                        # PSUM drain normalizes by 1/rowsum[q] (rcb is that
                        # row broadcast down the partitions), then splits
                        # into fp8 value + fp8 residual for DoubleRow.
                        for dt in range(DT):
                            pu = ps_u.tile([P, 512], F32, tag="pu",
                                           name=f"pu{dt}")
                            for kt in range(bext):
                                ou, os_ = u_offs[kt], e_offs[kt]
                                nc.tensor.matmul(
                                    pu[:, ou:512],
                                    xkd_sb[kt][:, dt * P:(dt + 1) * P],
                                    e_tiles[kt][:, ou - os_:512 - os_],
                                    start=(kt == 0), stop=(kt == bext - 1),
                                )
                            un = sp.tile([P, 512], BF16, tag="un", bufs=2,
                                         name=f"un{dt}")
                            nc.vector.tensor_mul(un[:], pu[:], rcb[:, 0:512])
                            ua = ut8a_sb[dt // 2][:, dt % 2:dt % 2 + 1,
                                                  q0:q0 + 512]
                            ub = ut8b_sb[dt // 2][:, dt % 2:dt % 2 + 1,
                                                  q0:q0 + 512]
                            nc.vector.tensor_copy(ua, un[:])
                            nc.vector.scalar_tensor_tensor(
                                ub, un[:], 1.0, ua, op0=MULT, op1=SUB)

                    def rowsums_block(s0, s1, e_tiles, e_offs):
                        # per-slot row sums (matmul with ones), reciprocal,
                        # then transpose the [q,1] reciprocals into one
                        # [1, 512] row and broadcast it down the partitions.
                        rcs = []
                        for ls in range(s0, s1):
                            lq = (ls - s0) * P
                            pm = ps_o.tile([P, 1], F32, tag="po",
                                           name=f"pm{ls}")
                            for kt in range(EXT[ls]):
                                el = e_tiles[kt][:, lq - e_offs[kt]:
                                                 lq - e_offs[kt] + P]
                                nc.tensor.matmul(pm[:], el, ones_sb[:],
                                                 start=(kt == 0),
                                                 stop=(kt == EXT[ls] - 1))
                            rc = sp.tile([P, 1], F32, tag="rc",
                                         name=f"rc{ls}")
                            nc.vector.reciprocal(rc[:], pm[:])
                            rcs.append(rc)
                        rT = sp.tile([1, 512], F32, tag="rT", bufs=1,
                                     name=f"rT{s0}")
                        for i, rc in enumerate(rcs):
                            prt = ps_o.tile([1, P], F32, tag="po",
                                            name=f"prt{s0}_{i}")
                            nc.tensor.matmul(prt[:], rc[:], id_sb[:],
                                             start=True, stop=True,
                                             is_transpose=True)
                            nc.vector.tensor_copy(rT[:, i * P:(i + 1) * P],
                                                  prt[:])
                        rcb = sp.tile([P, 512], F32, tag="rcb", bufs=2,
                                      name=f"rcb{s0}")
                        nc.gpsimd.partition_broadcast(rcb[:], rT[:])
                        return rcb

                    def out_block(s0, s1, last=False):
                        # out[q,d] = (U/rowsum) @ (128 Wv) / 128 + bv via
                        # fp8 DoubleRow with residual compensation
                        for ls in range(s0, s1):
                            ot = op.tile([P, D], F32, tag="ot")
                            pieces = [(0, 512), (512, 1024)]
                            if last and ls == s1 - 1:
                                # fine-grain the final tile so the trailing
                                # DVE scale + store DMA overlap the last
                                # matmul chains instead of serializing
                                pieces = [(0, 512), (512, 768),
                                          (768, 896), (896, 1024)]
                            for (lo, hi) in pieces:
                                po = ps_o.tile([P, 512], F32, tag="po",
                                               name=f"po{ls}_{lo}")
                                pv = po[:, 0:hi - lo]
                                qs = slice(ls * P, (ls + 1) * P)
                                terms = (
                                    [(ut8a_sb[t], wv8a_sb[t])
                                     for t in range(DT // 2)] +
                                    [(ut8b_sb[t], wv8a_sb[t])
                                     for t in range(DT // 2)] +
                                    [(ut8a_sb[t], wv8b_sb[t])
                                     for t in range(DT // 2)])
                                for i, (usrc, wsrc) in enumerate(terms):
                                    nc.tensor.matmul(
                                        pv,
                                        usrc[:, :, qs],
                                        wsrc[:, :, lo:hi],
                                        start=(i == 0),
                                        stop=(i == len(terms) - 1),
                                        perf_mode=DROW,
                                    )
                                nc.vector.scalar_tensor_tensor(
                                    ot[:, lo:hi], pv, 1.0 / 128.0,
                                    bv_sb[:, lo:hi], op0=MULT, op1=ADD)
                                nc.sync.dma_start(
                                    out=y[ls * P:(ls + 1) * P, lo:hi],
                                    in_=ot[:, lo:hi])

                    s0, s1, bext = BLK[0]
                    e0, eo0, uo0 = scores_block(s0, s1, bext, s0 * P)
                    rcb0 = rowsums_block(s0, s1, e0, eo0)
                    u_block(bext, s0 * P, e0, eo0, uo0, rcb0)

                    t0, t1, bext1 = BLK[1]
                    e1, eo1, uo1 = scores_block(t0, t1, bext1, t0 * P)
                    out_block(s0, s1)
                    rcb1 = rowsums_block(t0, t1, e1, eo1)
                    u_block(bext1, t0 * P, e1, eo1, uo1, rcb1)
                    out_block(t0, t1, last=True)

    nc.compile()
    return nc


def _get_nc():
    if "nc" not in _CACHE:
        _CACHE["nc"] = _build()
    return _CACHE["nc"]


def make_in_maps(X, Wq, bq, Wk, bk, Wv, bv):
    X = np.asarray(X, np.float32)
    Wq = np.asarray(Wq, np.float32)
    Wk = np.asarray(Wk, np.float32)
    Wv = np.ascontiguousarray(np.asarray(Wv, np.float32))
    bq = np.asarray(bq, np.float32)
    bv = np.asarray(bv, np.float32)

    G = Wq @ Wk.T                                # [D, D]
    # do-major retiling: gg[p, do*1024 + dd*128 + c] = G[dd*128+p, do*128+c]
    gg = np.ascontiguousarray(
        G.reshape(DT, P, DT, P).transpose(1, 2, 0, 3).reshape(P, DT * D))
    wkbq = Wk @ bq                               # [D]
    bvp = np.ascontiguousarray(np.broadcast_to(bv[None, :], (P, D)))

    # fp8 pair layout (x128 so entries sit in e4m3's normal range) plus
    # the fp8 residual: wv8a + wv8b ~ 128*Wv to ~0.4% relative.
    def pair8(w):
        return np.ascontiguousarray(
            w.reshape(DT // 2, 2, P, D).transpose(0, 2, 1, 3)
            .reshape((DT // 2) * P, 2 * D).astype(ml_dtypes.float8_e4m3fn))

    w128 = Wv * 128.0
    w8a = w128.astype(ml_dtypes.float8_e4m3fn).astype(np.float32)
    wv8a = pair8(w8a)
    wv8b = pair8(w128 - w8a)
    identity = np.eye(P, dtype=np.float32)

    masks = {}
    for h in (0, 1):
        m = np.zeros((QT, 2 * P, P), np.float32)
        for s in range(QT):
            qt = QTS[h][s]
            kk = (2 * s) * P + np.arange(2 * P)[:, None]
            qq = qt * P + np.arange(P)[None, :]
            m[s] = (kk <= qq)
        # [s, kk, q] -> [kk%128, s*256 + (kk//128)*128 + q]
        m2 = m.reshape(QT, 2, P, P).transpose(2, 0, 1, 3).reshape(P, QT * 2 * P)
        masks[h] = np.ascontiguousarray(m2.astype(ml_dtypes.bfloat16))

    in_maps = []
    for c in range(8):
        b, h = divmod(c, 2)
        Xb = X[b]
        xkt = np.ascontiguousarray(Xb.T)
        xkd = np.ascontiguousarray(Xb.astype(ml_dtypes.bfloat16))
        xq_rows = np.concatenate(
            [Xb[qt * P:(qt + 1) * P] for qt in QTS[h]], axis=0)
        xqt = np.ascontiguousarray(xq_rows.T)
        w = Xb @ wkbq                             # [S] additive k-bias
        wbp = np.ascontiguousarray(w.reshape(KT, P).T)   # [P, KT]
        in_maps.append({
            "xqt": xqt, "xkt": xkt, "g": gg, "xkd": xkd, "wv8a": wv8a,
            "wv8b": wv8b, "ident": identity, "wb": wbp, "bvp": bvp,
            "msk": masks[h],
        })
    return in_maps


def assemble(results):
    Y = np.empty((B, S, D), np.float32)
    for c in range(8):
        b, h = divmod(c, 2)
        yc = results[c]["y"]
        for s in range(QT):
            qt = QTS[h][s]
            Y[b, qt * P:(qt + 1) * P, :] = yc[s * P:(s + 1) * P, :]
    return Y


def kernel(X, Wq, bq, Wk, bk, Wv, bv):
    nc = _get_nc()
    in_maps = make_in_maps(X, Wq, bq, Wk, bk, Wv, bv)
    res = run_bass_kernel_spmd(nc, in_maps, core_ids=list(range(8)))
    return assemble(res.results)


# revision 36
# speedup vs baseline: 1.2905x; 1.0063x over previous
"""Causal single-head attention layer on 8 TRN2 NeuronCores.

Problem: X[4,2048,1024]; Q/K/V = X@W+b; scores = Q@K^T (no 1/sqrt(d));
causal mask; softmax; out = P@V.

Sharding: 2 cores per batch. Each core owns 8 query tiles (128 rows) of
its batch, folded for causal load balance:
  core h=0 -> global q-tiles (0,3,4,7,8,11,12,15)
  core h=1 -> global q-tiles (1,2,5,6,9,10,13,14)
Slot s on either core has causal extent <= 2s+2 k-tiles, so ONE uniform
program runs on all 8 cores; the exact causal boundary is a host-supplied
0/1 mask over the last two k-tiles of each slot.

Math restructuring:
  scores = (XqWq+bq)(XkWk+bk)^T
         = Xq G Xk^T + [q-only term] + w[k] + [const],  G = Wq Wk^T (host)
  q-only and const terms cancel in softmax; w[k] = Xk @ (Wk bq) (host)
  rides the per-partition bias slot of the Exp activation.
  The V projection is folded PAST the attention sum (associativity):
    out = (E @ Xk) @ Wv / rowsum + bv = U @ Wv / rowsum + bv
  so the [S,D]x[D,D] V projection (duplicated on both cores of a batch)
  is replaced by a per-core [1024,D]x[D,D] output projection.

On-device layout (contraction always on partitions):
  host passes X^T (xkt, f32r) for scores, X (xkd, bf16) for the U
  accumulation; scores^T[k,q] accumulate fp32 in PSUM; E = exp(scores^T
  + w[k]) in bf16 (w rides the Exp bias slot); U^T[d,q] = sum_k Xk^T E
  accumulates per d-tile in PSUM with causally narrowed moving dims
  (bf16 has no N>=256 restriction; fp32r scores clamp at N=256);
  row sums via matmul with ones; out[q,d] = (U@Wv)/sums + bv, Wv bf16.
  No max-subtraction needed: |scores| <= ~60 so exp stays in range.

Scheduling notes (PE pstate: any PE idle gap drops the clock for ~3us,
so the program is ordered to keep PE streaks long):
  - One flat PSUM pool set spans Qg + attention (Qg chains ride the U
    pool's ring) so there is no PSUM pool-transition barrier between
    the Qg copies and the first score matmul.
  - Qg runs as per-(do,c) chains, all c=0 chains first: their copies
    trail one chain behind, and scores of block 0 (which read only the
    c=0 half of Qg) start right after the c=1 chains with no wait.
  - DMA order = consumption order: (g|xq interleaved), xkt k-blocks
    0-1, wb+mask, xkd tiles 0-7, xkt k-blocks 2-3, xkd 8-15, wv, bvp.
  - PE order: Qg-c0, Qg-c1, [b0 scores, U, rowsums], [b1 scores],
    [b0 out], [b1 U, rowsums, out].
"""

import numpy as np
import ml_dtypes

import concourse.bass as bass  # noqa: F401
import concourse.mybir as mybir
from concourse import bacc
from concourse.bass_utils import run_bass_kernel_spmd
from concourse.tile import TileContext

F32 = mybir.dt.float32
F32R = mybir.dt.float32r
BF16 = mybir.dt.bfloat16
FP8 = mybir.dt.float8e4
EXP = mybir.ActivationFunctionType.Exp
MULT = mybir.AluOpType.mult
ADD = mybir.AluOpType.add
SUB = mybir.AluOpType.subtract
DROW = mybir.MatmulPerfMode.DoubleRow

B, S, D = 4, 2048, 1024
P = 128
DT = D // P          # 8 d-tiles
QT = 8               # q-tile slots per core
KT = S // P          # 16 k-tiles
EXT = [2 * s + 2 for s in range(QT)]   # uniform per-slot k-extent
BLK = [(0, 4, 8), (4, 8, 16)]          # (slot_lo, slot_hi, block k-extent)

QTS = {0: [0, 3, 4, 7, 8, 11, 12, 15], 1: [1, 2, 5, 6, 9, 10, 13, 14]}

_CACHE = {}


def _build(reps=1):
    nc = bacc.Bacc("TRN2", target_bir_lowering=False, debug=False, num_devices=8)
    xqt = nc.declare_dram_parameter("xqt", [D, QT * P], F32R, isOutput=False)
    xkt = nc.declare_dram_parameter("xkt", [D, S], F32R, isOutput=False)
    g = nc.declare_dram_parameter("g", [P, DT * D], F32R, isOutput=False)
    xkd = nc.declare_dram_parameter("xkd", [S, D], BF16, isOutput=False)
    wv8a = nc.declare_dram_parameter("wv8a", [(DT // 2) * P, 2 * D], FP8,
                                     isOutput=False)
    wv8b = nc.declare_dram_parameter("wv8b", [(DT // 2) * P, 2 * D], FP8,
                                     isOutput=False)
    ident = nc.declare_dram_parameter("ident", [P, P], F32, isOutput=False)
    wb = nc.declare_dram_parameter("wb", [P, KT], F32, isOutput=False)
    bvp = nc.declare_dram_parameter("bvp", [P, D], F32, isOutput=False)
    msk = nc.declare_dram_parameter("msk", [P, QT * 2 * P], BF16, isOutput=False)
    y = nc.declare_dram_parameter("y", [QT * P, D], F32, isOutput=True)

    with TileContext(nc) as tc:
      for _rep in range(reps):
        with tc.tile_pool(name="persist", bufs=1) as pp:
            # ---- persistent tiles ----
            xk_sb = [pp.tile([P, S], F32R, tag=f"xk{i}", name=f"xk{i}")
                     for i in range(DT)]
            qg_sb = [pp.tile([P, QT * P], F32R, tag=f"qg{i}", name=f"qg{i}")
                     for i in range(DT)]
            xkd_sb = [pp.tile([P, D], BF16, tag=f"xkd{i}", name=f"xkd{i}")
                      for i in range(KT)]
            wb_sb = pp.tile([P, KT], F32, tag="wb", name="wb_sb")
            bv_sb = pp.tile([P, D], F32, tag="bv", name="bv_sb")
            mask_sb = pp.tile([P, QT * 2 * P], BF16, tag="mask", name="mask_sb")
            ones_sb = pp.tile([P, 1], BF16, tag="ones", name="ones_sb")
            junk_sb = pp.tile([P, 512], BF16, tag="junk", name="junk_sb")
            id_sb = pp.tile([P, P], F32, tag="ident", name="id_sb")

            with (
                tc.tile_pool(name="pssc", bufs=2, space="PSUM") as ps_s,
                tc.tile_pool(name="psu", bufs=4, space="PSUM") as ps_u,
                tc.tile_pool(name="pso", bufs=2, space="PSUM") as ps_o,
            ):
                # ---- Qg phase: Qg^T[d2,q] = sum_d1 G[d1,d2] Xq^T[d1,q] ----
                with tc.tile_pool(name="qgpool", bufs=1) as qp:
                    xq_sb = [qp.tile([P, QT * P], F32R, tag=f"xq{i}",
                                     name=f"xq{i}") for i in range(DT)]
                    # g_sb[do] cols dd*128+c hold G[dd*128+p, do*128+c]
                    # (host-retiled so each do-block is one large DMA).
                    g_sb = [qp.tile([P, D], F32R, tag=f"g{i}", name=f"g{i}")
                            for i in range(DT)]
                    # c=0 chains need only the first halves of xq, so land
                    # those, then the g blocks, then the second halves.
                    for dd in range(DT):
                        nc.sync.dma_start(out=xq_sb[dd][:, 0:512],
                                          in_=xqt[dd * P:(dd + 1) * P, 0:512])
                    for do in range(DT):
                        nc.sync.dma_start(out=g_sb[do][:],
                                          in_=g[:, do * D:(do + 1) * D])
                    for dd in range(DT):
                        nc.sync.dma_start(out=xq_sb[dd][:, 512:1024],
                                          in_=xqt[dd * P:(dd + 1) * P,
                                                  512:1024])
                    # attention inputs stream in behind the Qg inputs, in
                    # consumption order (k-block-major for the scores).
                    for kb in range(2):
                        for dd in range(DT):
                            nc.sync.dma_start(
                                out=xk_sb[dd][:, kb * 512:(kb + 1) * 512],
                                in_=xkt[dd * P:(dd + 1) * P,
                                        kb * 512:(kb + 1) * 512])
                    nc.sync.dma_start(out=wb_sb[:], in_=wb[:])
                    nc.sync.dma_start(out=mask_sb[:], in_=msk[:])
                    nc.sync.dma_start(out=id_sb[:], in_=ident[:])
                    for kt in range(8):
                        nc.sync.dma_start(out=xkd_sb[kt][:],
                                          in_=xkd[kt * P:(kt + 1) * P, :])
                    for kb in range(2, 4):
                        for dd in range(DT):
                            nc.sync.dma_start(
                                out=xk_sb[dd][:, kb * 512:(kb + 1) * 512],
                                in_=xkt[dd * P:(dd + 1) * P,
                                        kb * 512:(kb + 1) * 512])
                    for kt in range(8, KT):
                        nc.sync.dma_start(out=xkd_sb[kt][:],
                                          in_=xkd[kt * P:(kt + 1) * P, :])
                    nc.sync.dma_start(out=bv_sb[:], in_=bvp[:])
                    nc.gpsimd.memset(ones_sb[:], 1.0)
                    nc.gpsimd.memset(junk_sb[:], 0.0)

                    # PE warm-up: dummy matmuls spanning the initial DMA
                    # wait (~10us) so the p-state ramp completes before the
                    # first real chain; the PE would otherwise idle here and
                    # every post-gap instruction pays the slow-clock ramp.
                    for w in range(38):
                        pw = ps_o.tile([1, 512], F32, tag="po",
                                       name=f"warm{w}")
                        nc.tensor.matmul(pw[:], ones_sb[:], junk_sb[:],
                                         start=True, stop=True)

                    # Qg chains ride the psu ring: no fresh PSUM pool, so
                    # the first score matmul has no pool-transition wait.
                    for c in range(2):
                        for do in range(DT):
                            pq = ps_u.tile([P, 512], F32, tag="pu",
                                           name=f"pq{c}_{do}")
                            for dd in range(DT):
                                nc.tensor.matmul(
                                    pq[:],
                                    g_sb[do][:, dd * P:(dd + 1) * P],
                                    xq_sb[dd][:, c * 512:(c + 1) * 512],
                                    start=(dd == 0), stop=(dd == DT - 1),
                                )
                            nc.vector.tensor_copy(
                                qg_sb[do][:, c * 512:(c + 1) * 512], pq[:])

                # ---- Attention ----
                with (
                    tc.tile_pool(name="attn", bufs=1) as ap,
                    tc.tile_pool(name="estage", bufs=18) as ep,
                    tc.tile_pool(name="ostage", bufs=2) as op,
                    tc.tile_pool(name="small", bufs=4) as sp,
                ):
                    # fp8 pair tiles for the DoubleRow out-projection with
                    # residual compensation: Ua/Wa are fp8 roundings of
                    # U/rowsum and 128*Wv; Ub/Wb their fp8 residuals.
                    # out = Ua@Wa + Ub@Wa + Ua@Wb (UbWb ~ 0.1%, dropped).
                    wv8a_sb = [ap.tile([P, 2, D], FP8, tag=f"wva{i}",
                                       name=f"wva{i}") for i in range(DT // 2)]
                    wv8b_sb = [ap.tile([P, 2, D], FP8, tag=f"wvb{i}",
                                       name=f"wvb{i}") for i in range(DT // 2)]
                    ut8a_sb = [ap.tile([P, 2, QT * P], FP8, tag=f"uta{i}",
                                       name=f"uta{i}") for i in range(DT // 2)]
                    ut8b_sb = [ap.tile([P, 2, QT * P], FP8, tag=f"utb{i}",
                                       name=f"utb{i}") for i in range(DT // 2)]
                    for t in range(DT // 2):
                        nc.sync.dma_start(out=wv8a_sb[t][:],
                                          in_=wv8a[t * P:(t + 1) * P, :])
                    for t in range(DT // 2):
                        nc.sync.dma_start(out=wv8b_sb[t][:],
                                          in_=wv8b[t * P:(t + 1) * P, :])

                    def scores_block(s0, s1, bext, q0):
                        e_tiles, e_offs, u_offs = [], [], []
                        for kt in range(bext):
                            # slots below ls_min never read k-tile kt
                            # (causal): narrow the moving dim, fp32r keeps
                            # N >= 256 (below which it slows 4x).
                            ls_min = kt // 2
                            offu = max(0, ls_min - s0) * P
                            offs = min(offu, 256)
                            n = 512 - offs
                            pscore = ps_s.tile([P, 512], F32, tag="sc")
                            for dd in range(DT):
                                nc.tensor.matmul(
                                    pscore[:, 0:n],
                                    xk_sb[dd][:, kt * P:(kt + 1) * P],
                                    qg_sb[dd][:, q0 + offs:q0 + 512],
                                    start=(dd == 0), stop=(dd == DT - 1),
                                )
                            et = ep.tile([P, 512], BF16, tag="E")
                            # E = exp(scores^T + w[k]) (w in the bias slot)
                            nc.scalar.activation(et[:, 0:n], pscore[:, 0:n],
                                                 EXP, bias=wb_sb[:, kt:kt + 1])
                            e_tiles.append(et)
                            e_offs.append(offs)
                            u_offs.append(offu)
                            # causal boundary mask on each slot's two
                            # diagonal k-tiles, applied eagerly so the U
                            # accumulation can batch whole q-blocks.
                            for ls in range(s0, s1):
                                if kt == EXT[ls] - 2 or kt == EXT[ls] - 1:
                                    j = kt - (EXT[ls] - 2)
                                    lo = (ls - s0) * P - offs
                                    nc.vector.tensor_mul(
                                        et[:, lo:lo + P],
                                        et[:, lo:lo + P],
                                        mask_sb[:, (2 * ls + j) * P:
                                                (2 * ls + j + 1) * P],
                                    )
                        return e_tiles, e_offs, u_offs

                    def u_block(bext, q0, e_tiles, e_offs, u_offs, rcb):
                        # U^T[d,q] = sum_k Xk[k,d]^T E[k,q].  Column ranges
                        # narrow monotonically with kt and nest inside the
                        # kt=0 full-width start=True write, so each column
                        # accumulates exactly its causal k-extent.  The
                        # PSUM drain normalizes by 1/rowsum[q] (rcb is that
                        # row broadcast down the partitions), then splits
                        # into fp8 value + fp8 residual for DoubleRow.
                        for dt in range(DT):
                            pu = ps_u.tile([P, 512], F32, tag="pu",
                                           name=f"pu{dt}")
                            for kt in range(bext):
                                ou, os_ = u_offs[kt], e_offs[kt]
                                nc.tensor.matmul(
                                    pu[:, ou:512],
                                    xkd_sb[kt][:, dt * P:(dt + 1) * P],
                                    e_tiles[kt][:, ou - os_:512 - os_],
                                    start=(kt == 0), stop=(kt == bext - 1),
                                )
                            un = sp.tile([P, 512], BF16, tag="un", bufs=2,
                                         name=f"un{dt}")
                            nc.vector.tensor_mul(un[:], pu[:], rcb[:, 0:512])
                            ua = ut8a_sb[dt // 2][:, dt % 2:dt % 2 + 1,
                                                  q0:q0 + 512]
                            ub = ut8b_sb[dt // 2][:, dt % 2:dt % 2 + 1,
                                                  q0:q0 + 512]
                            nc.vector.tensor_copy(ua, un[:])
                            nc.vector.scalar_tensor_tensor(
                                ub, un[:], 1.0, ua, op0=MULT, op1=SUB)

                    def rowsums_block(s0, s1, e_tiles, e_offs):
                        # per-slot row sums (matmul with ones), reciprocal,
                        # then transpose the [q,1] reciprocals into one
                        # [1, 512] row and broadcast it down the partitions.
                        rcs = []
                        for ls in range(s0, s1):
                            lq = (ls - s0) * P
                            pm = ps_o.tile([P, 1], F32, tag="po",
                                           name=f"pm{ls}")
                            for kt in range(EXT[ls]):
                                el = e_tiles[kt][:, lq - e_offs[kt]:
                                                 lq - e_offs[kt] + P]
                                nc.tensor.matmul(pm[:], el, ones_sb[:],
                                                 start=(kt == 0),
                                                 stop=(kt == EXT[ls] - 1))
                            rc = sp.tile([P, 1], F32, tag="rc",
                                         name=f"rc{ls}")
                            nc.vector.reciprocal(rc[:], pm[:])
                            rcs.append(rc)
                        rT = sp.tile([1, 512], F32, tag="rT", bufs=1,
                                     name=f"rT{s0}")
                        for i, rc in enumerate(rcs):
                            prt = ps_o.tile([1, P], F32, tag="po",
                                            name=f"prt{s0}_{i}")
                            nc.tensor.matmul(prt[:], rc[:], id_sb[:],
                                             start=True, stop=True,
                                             is_transpose=True)
                            nc.vector.tensor_copy(rT[:, i * P:(i + 1) * P],
                                                  prt[:])
                        rcb = sp.tile([P, 512], F32, tag="rcb", bufs=2,
                                      name=f"rcb{s0}")
                        nc.gpsimd.partition_broadcast(rcb[:], rT[:])
                        return rcb

                    def out_block(s0, s1, last=False):
                        # out[q,d] = (U/rowsum) @ (128 Wv) / 128 + bv via
                        # fp8 DoubleRow with residual compensation
                        for ls in range(s0, s1):
                            ot = op.tile([P, D], F32, tag="ot")
                            pieces = [(0, 512), (512, 1024)]
                            if last and ls == s1 - 1:
                                # fine-grain the final tile so the trailing
                                # DVE scale + store DMA overlap the last
                                # matmul chains instead of serializing
                                pieces = [(0, 512), (512, 768),
                                          (768, 896), (896, 1024)]
                            for (lo, hi) in pieces:
                                po = ps_o.tile([P, 512], F32, tag="po",
                                               name=f"po{ls}_{lo}")
                                pv = po[:, 0:hi - lo]
                                qs = slice(ls * P, (ls + 1) * P)
                                terms = (
                                    [(ut8a_sb[t], wv8a_sb[t])
                                     for t in range(DT // 2)] +
                                    [(ut8b_sb[t], wv8a_sb[t])
                                     for t in range(DT // 2)] +
                                    [(ut8a_sb[t], wv8b_sb[t])
                                     for t in range(DT // 2)])
                                for i, (usrc, wsrc) in enumerate(terms):
                                    nc.tensor.matmul(
                                        pv,
                                        usrc[:, :, qs],
                                        wsrc[:, :, lo:hi],
                                        start=(i == 0),
                                        stop=(i == len(terms) - 1),
                                        perf_mode=DROW,
                                    )
                                nc.vector.scalar_tensor_tensor(
                                    ot[:, lo:hi], pv, 1.0 / 128.0,
                                    bv_sb[:, lo:hi], op0=MULT, op1=ADD)
                                nc.sync.dma_start(
                                    out=y[ls * P:(ls + 1) * P, lo:hi],
                                    in_=ot[:, lo:hi])

                    s0, s1, bext = BLK[0]
                    e0, eo0, uo0 = scores_block(s0, s1, bext, s0 * P)
                    rcb0 = rowsums_block(s0, s1, e0, eo0)
                    u_block(bext, s0 * P, e0, eo0, uo0, rcb0)

                    t0, t1, bext1 = BLK[1]
                    e1, eo1, uo1 = scores_block(t0, t1, bext1, t0 * P)
                    # hold one block-0 out slot back so it fills the join
                    # where block-1's trailing U copies gate block-1 outs
                    out_block(s0, s1 - 1)
                    rcb1 = rowsums_block(t0, t1, e1, eo1)
                    u_block(bext1, t0 * P, e1, eo1, uo1, rcb1)
                    out_block(s1 - 1, s1)
                    out_block(t0, t1, last=True)

    nc.compile()
    return nc


def _get_nc():
    if "nc" not in _CACHE:
        _CACHE["nc"] = _build()
    return _CACHE["nc"]


def make_in_maps(X, Wq, bq, Wk, bk, Wv, bv):
    X = np.asarray(X, np.float32)
    Wq = np.asarray(Wq, np.float32)
    Wk = np.asarray(Wk, np.float32)
    Wv = np.ascontiguousarray(np.asarray(Wv, np.float32))
    bq = np.asarray(bq, np.float32)
    bv = np.asarray(bv, np.float32)

    G = Wq @ Wk.T                                # [D, D]
    # do-major retiling: gg[p, do*1024 + dd*128 + c] = G[dd*128+p, do*128+c]
    gg = np.ascontiguousarray(
        G.reshape(DT, P, DT, P).transpose(1, 2, 0, 3).reshape(P, DT * D))
    wkbq = Wk @ bq                               # [D]
    bvp = np.ascontiguousarray(np.broadcast_to(bv[None, :], (P, D)))

    # fp8 pair layout (x128 so entries sit in e4m3's normal range) plus
    # the fp8 residual: wv8a + wv8b ~ 128*Wv to ~0.4% relative.
    def pair8(w):
        return np.ascontiguousarray(
            w.reshape(DT // 2, 2, P, D).transpose(0, 2, 1, 3)
            .reshape((DT // 2) * P, 2 * D).astype(ml_dtypes.float8_e4m3fn))

    w128 = Wv * 128.0
    w8a = w128.astype(ml_dtypes.float8_e4m3fn).astype(np.float32)
    wv8a = pair8(w8a)
    wv8b = pair8(w128 - w8a)
    identity = np.eye(P, dtype=np.float32)

    masks = {}
    for h in (0, 1):
        m = np.zeros((QT, 2 * P, P), np.float32)
        for s in range(QT):
            qt = QTS[h][s]
            kk = (2 * s) * P + np.arange(2 * P)[:, None]
            qq = qt * P + np.arange(P)[None, :]
            m[s] = (kk <= qq)
        # [s, kk, q] -> [kk%128, s*256 + (kk//128)*128 + q]
        m2 = m.reshape(QT, 2, P, P).transpose(2, 0, 1, 3).reshape(P, QT * 2 * P)
        masks[h] = np.ascontiguousarray(m2.astype(ml_dtypes.bfloat16))

    in_maps = []
    for c in range(8):
        b, h = divmod(c, 2)
        Xb = X[b]
        xkt = np.ascontiguousarray(Xb.T)
        xkd = np.ascontiguousarray(Xb.astype(ml_dtypes.bfloat16))
        xq_rows = np.concatenate(
            [Xb[qt * P:(qt + 1) * P] for qt in QTS[h]], axis=0)
        xqt = np.ascontiguousarray(xq_rows.T)
        w = Xb @ wkbq                             # [S] additive k-bias
        wbp = np.ascontiguousarray(w.reshape(KT, P).T)   # [P, KT]
        in_maps.append({
            "xqt": xqt, "xkt": xkt, "g": gg, "xkd": xkd, "wv8a": wv8a,
            "wv8b": wv8b, "ident": identity, "wb": wbp, "bvp": bvp,
            "msk": masks[h],
        })
    return in_maps


def assemble(results):
    Y = np.empty((B, S, D), np.float32)
    for c in range(8):
        b, h = divmod(c, 2)
        yc = results[c]["y"]
        for s in range(QT):
            qt = QTS[h][s]
            Y[b, qt * P:(qt + 1) * P, :] = yc[s * P:(s + 1) * P, :]
    return Y


def kernel(X, Wq, bq, Wk, bk, Wv, bv):
    nc = _get_nc()
    in_maps = make_in_maps(X, Wq, bq, Wk, bk, Wv, bv)
    res = run_bass_kernel_spmd(nc, in_maps, core_ids=list(range(8)))
    return assemble(res.results)
